# revision 1
# baseline (speedup 1.0000x reference)
"""Trainium2 Bass kernel for nn_EquivariantPerturbationTransform.

Reference computation (N=6000 genes, D=256, H=8 heads, P=128 perturbations,
B=16 batches):
  q = H @ Wq.T ; k,v from gathered perturbation rows
  scores[h,n,p] shared across batches; per-batch mask over p (ragged)
  attn_out[b] = softmax-masked attention -> out proj (zeroed for empty batches)
  x = LN1(H + attn_out); out = LN2(x + gelu(x@W1.T)@W2.T)

Strategy:
  - Sequence-parallel over 8 cores: N padded to 6144, 768 query rows/core,
    all B batches per core. H_genes/params replicated (small), so per-core
    HBM traffic is ~2.5MB of inputs + 12.3MB output.
  - batch_assignment is sorted -> each batch owns a CONTIGUOUS p-range. The
    128 perturbations are cut into eight 16-row blocks; a block-diagonal
    value matrix V_bd[(h,p16), (slot,h',e)] per block turns the masked
    per-batch, per-head attention*V contraction into plain 128-K matmuls
    producing row-layout context for one batch-slot at a time (never
    materializes [B,h,N,P] logits, no partition-offset matmul outputs --
    fp32r matmuls only accept column tile-position 0 on this compiler).
  - exp() without max-subtraction: scores are O(10) here so fp32 exp is safe
    (ratios exact); softmax denominators via one mask-matmul per head,
    transposed per-head to row layout [n, (h,b)] and applied as a
    broadcast-AP multiply; empty batches get +1 denominators and skip
    attention entirely at program-build time.
  - fp32r everywhere on the PE (full 1 cycle/row at moving-N>=256, ~1e-4
    matmul error vs 2e-3 for bf16); fp32 accumulation in PSUM.
"""

import os
import sys

sys.path.insert(0, "/opt/trn_rl_repo")

import numpy as np

import concourse.bass as bass
from concourse import mybir
from concourse.tile import TileContext

F32 = mybir.dt.float32
F32R = mybir.dt.float32r
AF = mybir.ActivationFunctionType

N, D, H, P, B = 6000, 256, 8, 128, 16
DH = D // H  # 32
NCORES = 8
NPAD = 6144          # 8 * 768
NG = NPAD // NCORES  # 768 rows per core
NT = NG // 128       # 6 row-tiles per core
NCH = 2              # moving-dim chunks
CH = NG // NCH       # 384 (>=256 keeps fp32r at full rate)
EPS = 1e-5
GW = 16              # perturbation block width
NGRP = P // GW       # 8 blocks


def _split_waits(nc, max_waits=1):
    """The neuronxcc/walrus build in this container rejects instructions with
    more than one sync-wait condition. Hoist excess waits onto NoOps injected
    just before, on the same engine (semantically identical)."""
    n_split = 0
    for f in nc.m.functions:
        for bb in f.blocks:
            new_list = []
            for ins in bb.instructions:
                si = getattr(ins, "sync_info", None)
                if si is not None and si.on_wait and len(si.on_wait) > max_waits:
                    waits = list(si.on_wait)
                    excess, keep = waits[:-max_waits], waits[-max_waits:]
                    for i in range(0, len(excess), max_waits):
                        chunk = excess[i : i + max_waits]
                        nop = mybir.InstNoOp(name=f"{ins.name}-ws{i}", ins=[], outs=[])
                        nop.engine = ins.engine
                        nop.sync_info = mybir.SyncInfo(on_wait=chunk, on_update=[])
                        new_list.append(nop)
                        n_split += 1
                    si.on_wait = keep
                new_list.append(ins)
            bb.instructions = new_list
    return n_split


def _build_program(counts, groups, contribs, flags):
    """Build the per-core SPMD Bass program.

    groups[g]   = list of (b, p_lo, p_len) for batches intersecting block g
    contribs[b] = list of (g, slot_idx) covering batch b's p-range
    """
    (use_bq, use_bk, use_bv, use_bo, use_b1, use_b2,
     use_g1, use_b1ln, use_g2, use_b2ln) = flags
    nc = bass.Bass()

    # ---- DRAM parameters -------------------------------------------------
    hg_row = nc.declare_dram_parameter("hg_row", [NG, D], F32, isOutput=False)
    hg_t = nc.declare_dram_parameter("hg_t", [D, NG], F32R, isOutput=False)
    hp_t = nc.declare_dram_parameter("hp_t", [D, P], F32R, isOutput=False)
    m01bd = nc.declare_dram_parameter("m01bd", [NGRP, 128, 128], F32R, isOutput=False)
    emptyp = nc.declare_dram_parameter("emptyp", [128, 1], F32, isOutput=False)
    ident = nc.declare_dram_parameter("ident", [128, 128], F32, isOutput=False)
    identr = nc.declare_dram_parameter("identr", [128, 128], F32R, isOutput=False)
    identb = nc.declare_dram_parameter("identb", [128, 32], F32, isOutput=False)
    smax = max(1, max(len(g) for g in groups))
    zeros_r = nc.declare_dram_parameter("zeros_r", [128, smax * D], F32R, isOutput=False)
    wq_t = nc.declare_dram_parameter("wq_t", [D, D], F32R, isOutput=False)
    wk_t = nc.declare_dram_parameter("wk_t", [D, D], F32R, isOutput=False)
    wv_t = nc.declare_dram_parameter("wv_t", [D, D], F32R, isOutput=False)
    wo_t = nc.declare_dram_parameter("wo_t", [D, D], F32R, isOutput=False)
    w1_t = nc.declare_dram_parameter("w1_t", [D, 4 * D], F32R, isOutput=False)
    w2_t = nc.declare_dram_parameter("w2_t", [4 * D, D], F32R, isOutput=False)
    bias_kv = nc.declare_dram_parameter("bias_kv", [D, 2], F32, isOutput=False)
    bq_col = nc.declare_dram_parameter("bq_col", [D, 1], F32, isOutput=False)
    bo_mask = nc.declare_dram_parameter("bo_mask", [D, B], F32, isOutput=False)
    b1_col = nc.declare_dram_parameter("b1_col", [4 * D, 1], F32, isOutput=False)
    b2_col = nc.declare_dram_parameter("b2_col", [D, 1], F32, isOutput=False)
    ln1_col = nc.declare_dram_parameter("ln1_col", [D, 2], F32, isOutput=False)
    gb_row = nc.declare_dram_parameter("gb_row", [4, D], F32, isOutput=False)
    out = nc.declare_dram_parameter("out", [B, NG, D], F32, isOutput=True)

    s_attn = 1.0 / float(np.sqrt(DH))

    with TileContext(nc) as tc, nc.allow_low_precision(
            reason="fp32r is a deliberate rounding of matmul inputs"):
        import contextlib

        cstack = contextlib.ExitStack()
        consts = cstack.enter_context(tc.tile_pool(name="consts", bufs=1))

        # ---- load constants -------------------------------------------
        hgr_sb = []
        for t in range(NT):
            tl = consts.tile([128, D], F32, tag=f"hgr{t}", name=f"hgr{t}")
            nc.sync.dma_start(out=tl[:], in_=hg_row[t * 128 : (t + 1) * 128, :])
            hgr_sb.append(tl)
        hgt_sb = []
        for kk in range(2):
            tl = consts.tile([128, NG], F32R, tag=f"hgt{kk}", name=f"hgt{kk}")
            nc.sync.dma_start(out=tl[:], in_=hg_t[kk * 128 : (kk + 1) * 128, :])
            hgt_sb.append(tl)
        hpt_sb = []
        for kk in range(2):
            tl = consts.tile([128, P], F32R, tag=f"hpt{kk}", name=f"hpt{kk}")
            nc.sync.dma_start(out=tl[:], in_=hp_t[kk * 128 : (kk + 1) * 128, :])
            hpt_sb.append(tl)

        def load_w(name, ap, rows, cols, dt=F32):
            tiles = []
            for kk in range(rows // 128):
                tl = consts.tile([128, cols], dt, tag=f"{name}{kk}", name=f"{name}{kk}")
                nc.sync.dma_start(out=tl[:], in_=ap[kk * 128 : (kk + 1) * 128, :])
                tiles.append(tl)
            return tiles

        wq_sb = load_w("wq", wq_t, D, D, dt=F32R)
        wk_sb = load_w("wk", wk_t, D, D, dt=F32R)
        wv_sb = load_w("wv", wv_t, D, D, dt=F32R)
        wo_sb = load_w("wo", wo_t, D, D, dt=F32R)
        w1_sb = load_w("w1", w1_t, D, 4 * D, dt=F32R)
        w2_sb = load_w("w2", w2_t, 4 * D, D, dt=F32R)

        m01bd_sb = []
        for g in range(NGRP):
            tl = consts.tile([128, 128], F32R, tag=f"m01bd{g}", name=f"m01bd{g}")
            nc.sync.dma_start(out=tl[:], in_=m01bd[g, :, :])
            m01bd_sb.append(tl)
        empty_sb = consts.tile([128, 1], F32, tag="empty", name="empty")
        nc.sync.dma_start(out=empty_sb[:], in_=emptyp[:, :])
        ident_sb = consts.tile([128, 128], F32, tag="ident", name="ident")
        nc.sync.dma_start(out=ident_sb[:], in_=ident[:, :])
        identr_sb = consts.tile([128, 128], F32R, tag="identr", name="identr")
        nc.sync.dma_start(out=identr_sb[:], in_=identr[:, :])
        # per-32-block identity so transposes of partition-offset slices can
        # use an identity operand starting at the same partition
        identb_sb = consts.tile([128, 32], F32, tag="identb", name="identb")
        nc.sync.dma_start(out=identb_sb[:], in_=identb[:, :])
        eps_sb = consts.tile([128, 1], F32, tag="eps", name="eps")
        nc.vector.memset(eps_sb[:], EPS)

        bkv_sb = None
        if use_bk or use_bv:
            bkv_sb = load_w("bkv", bias_kv, D, 2)
        bq_sb = load_w("bq", bq_col, D, 1) if use_bq else None
        bo_sb = load_w("bo", bo_mask, D, B) if use_bo else None
        b1_sb = load_w("b1", b1_col, 4 * D, 1) if use_b1 else None
        b2_sb = load_w("b2", b2_col, D, 1) if use_b2 else None
        # ln1 affine as [D,1] columns for the T-layout residual (general path)
        ln1_sb = load_w("ln1c", ln1_col, D, 2) if (use_g1 or use_b1ln) else None
        # broadcast [1,D] rows across 128 partitions for general ln2-affine /
        # v-bias paths (skipped when trivial)
        gbr_sb = None
        if use_g2 or use_b2ln or use_bv:
            gbr_sb = consts.tile([128, 4, D], F32, tag="gbr", name="gbr")
            nc.gpsimd.dma_start(out=gbr_sb[:], in_=gb_row[:, :].to_broadcast((128, 4, D)))

        # persistent activation tiles
        qT_sb = [consts.tile([128, NG], F32R, tag=f"qT{i}", name=f"qT{i}") for i in range(2)]
        kT_sb = [consts.tile([128, P], F32R, tag=f"kT{i}", name=f"kT{i}") for i in range(2)]
        v_sb = consts.tile([P, D], F32R, tag="v", name="v")
        # E regrouped per perturbation block: Eg[g][(h, p16), n]
        Eg = [consts.tile([128, NG], F32R, tag=f"Eg{g}", name=f"Eg{g}")
              for g in range(NGRP)]
        # block-diagonal masked values: vbd[g][(h, p16), (slot, h', e)]
        vbd = [consts.tile([128, max(1, len(groups[g])) * D], F32R,
                           tag=f"vbd{g}", name=f"vbd{g}") for g in range(NGRP)]
        # softmax denominators: packed [(h,b), n], then row layout [n, (h,b)]
        denp = consts.tile([128, NG], F32, tag="denp", name="denp")
        rden_row = consts.tile([128, NT, 128], F32, tag="rden_row", name="rden_row")

        # ================= Phase A: shared projections ==================
        with tc.tile_pool(name="psA", bufs=4, space="PSUM") as psA, \
             tc.tile_pool(name="psD", bufs=2, space="PSUM") as psD, \
             tc.tile_pool(name="etpool", bufs=1) as etpool:
            Et = etpool.tile([128, H, NG], F32R, tag="Et", name="Et")
            # qT [D, NG] = Wq^T-stationary applied to hg_t
            for m in range(2):
                for c in range(NCH):
                    ps = psA.tile([128, CH], F32, tag="ps", name="ps")
                    for kk in range(2):
                        nc.tensor.matmul(
                            ps[:],
                            wq_sb[kk][:, m * 128 : (m + 1) * 128],
                            hgt_sb[kk][:, c * CH : (c + 1) * CH],
                            start=(kk == 0), stop=(kk == 1),
                        )
                    if use_bq:
                        nc.scalar.activation(
                            qT_sb[m][:, c * CH : (c + 1) * CH], ps[:], AF.Identity,
                            bias=bq_sb[m][:, 0:1])
                    else:
                        nc.scalar.activation(
                            qT_sb[m][:, c * CH : (c + 1) * CH], ps[:], AF.Copy)

            # kT [D, P]
            for m in range(2):
                ps = psA.tile([128, P], F32, tag="ps", name="ps")
                for kk in range(2):
                    nc.tensor.matmul(
                        ps[:], wk_sb[kk][:, m * 128 : (m + 1) * 128],
                        hpt_sb[kk][:], start=(kk == 0), stop=(kk == 1))
                if use_bk:
                    nc.scalar.activation(kT_sb[m][:], ps[:], AF.Identity,
                                         bias=bkv_sb[m][:, 0:1])
                else:
                    nc.scalar.activation(kT_sb[m][:], ps[:], AF.Copy)

            # v row-layout [P, D]
            ps_v = psA.tile([P, D], F32, tag="ps", name="psv")
            for kk in range(2):
                nc.tensor.matmul(ps_v[:], hpt_sb[kk][:], wv_sb[kk][:],
                                 start=(kk == 0), stop=(kk == 1))
            if use_bv:
                nc.vector.tensor_add(v_sb[:], ps_v[:], gbr_sb[:P, 3, :])
            else:
                nc.vector.tensor_copy(out=v_sb[:], in_=ps_v[:])

            # E^T per head: exp(s * k_h @ q_h^T)  -> Et[p, h, n]
            for h in range(H):
                kt = kT_sb[h // 4]
                for c in range(NCH):
                    ps = psA.tile([128, CH], F32, tag="ps", name="ps")
                    nc.tensor.matmul(
                        ps[:],
                        kt[(h % 4) * DH : (h % 4 + 1) * DH, :],
                        qT_sb[h // 4][(h % 4) * DH : (h % 4 + 1) * DH,
                                      c * CH : (c + 1) * CH],
                        start=True, stop=True,
                        tile_position=((h % 4) * DH, 0))
                    nc.scalar.activation(Et[:, h, c * CH : (c + 1) * CH],
                                         ps[:], AF.Exp, scale=s_attn)

            # regroup E into per-block layout (partition moves via DMA)
            for g in range(NGRP):
                for h in range(H):
                    nc.sync.dma_start(
                        out=Eg[g][h * GW : (h + 1) * GW, :],
                        in_=Et[g * GW : (g + 1) * GW, h, :])

            # block-diagonal masked values (zero-fill via DMA: memset can't
            # write fp32r-typed tiles on this compiler)
            for g in range(NGRP):
                if groups[g]:
                    nc.sync.dma_start(
                        out=vbd[g][:],
                        in_=zeros_r[:, : len(groups[g]) * D])
                for s, (b, p_lo, p_len) in enumerate(groups[g]):
                    for h in range(H):
                        po = p_lo - g * GW
                        nc.sync.dma_start(
                            out=vbd[g][h * GW + po : h * GW + po + p_len,
                                       s * D + h * DH : s * D + (h + 1) * DH],
                            in_=v_sb[p_lo : p_lo + p_len,
                                     h * DH : (h + 1) * DH])

            # denominators, packed layout: denp[(h,b), n] via block-diagonal
            # mask matmuls accumulated over the 8 perturbation blocks
            for c in range(NCH):
                psd = psD.tile([128, CH], F32, tag="psd", name="psd")
                for g in range(NGRP):
                    nc.tensor.matmul(
                        psd[:], m01bd_sb[g][:],
                        Eg[g][:, c * CH : (c + 1) * CH],
                        start=(g == 0), stop=(g == NGRP - 1))
                # +1 on empty batches so the reciprocal is finite
                nc.scalar.activation(
                    denp[:, c * CH : (c + 1) * CH],
                    psd[:], AF.Identity, bias=empty_sb[:, 0:1])
            nc.vector.reciprocal(out=denp[:], in_=denp[:])
            # transpose reciprocal denominators to row layout [n, (h,b)],
            # two heads (32 rows) per transpose to stay 32-aligned
            for t in range(NT):
                for hp in range(4):
                    psr = psD.tile([128, 32], F32, tag="psd", name="psr")
                    nc.tensor.transpose(
                        psr[:], denp[hp * 32 : (hp + 1) * 32,
                                     t * 128 : (t + 1) * 128],
                        identb_sb[hp * 32 : (hp + 1) * 32, :],
                        tile_position=(hp * 32, 0))
                    nc.scalar.activation(
                        rden_row[:, t, hp * 32 : (hp + 1) * 32],
                        psr[:], AF.Copy)

        # ================= Phase B: per-batch back half =================
        work = cstack.enter_context(tc.tile_pool(name="work", bufs=2))
        h1pool = cstack.enter_context(tc.tile_pool(name="h1p", bufs=1))
        ps_mm = cstack.enter_context(tc.tile_pool(name="ps_mm", bufs=4, space="PSUM"))
        ps_tr = cstack.enter_context(tc.tile_pool(name="ps_tr", bufs=4, space="PSUM"))

        for b in range(B):
            Lb = int(counts[b]) if b < len(counts) else 0
            ctxT = None
            if Lb > 0:
                # --- attention context: block-diag matmuls give row-layout
                # ctx per batch; normalize with broadcast-AP multiply; PE
                # transpose into [(h,e), n] for the projection.
                ctxT = work.tile([128, 2, NG], F32R, tag="ctxT", name="ctxT")
                for t in range(NT):
                    psc = ps_mm.tile([128, D], F32, tag="mm", name="mmc")
                    cl = contribs[b]
                    for i, (g, s) in enumerate(cl):
                        nc.tensor.matmul(
                            psc[:],
                            Eg[g][:, t * 128 : (t + 1) * 128],
                            vbd[g][:, s * D : (s + 1) * D],
                            start=(i == 0), stop=(i == len(cl) - 1))
                    # multiply by 1/denom[n, h] (free-dim broadcast over e)
                    rr = rden_row[:, t, :]
                    rbc = bass.AP(tensor=rr.tensor, offset=rr.offset + b,
                                  ap=[rr.ap[0], [GW, H], [0, DH]])
                    ctxr = work.tile([128, H, DH], F32R, tag="ctxr", name="ctxr")
                    nc.vector.tensor_mul(
                        ctxr[:],
                        psc[:].rearrange("p (h e) -> p h e", h=H), rbc)
                    pst = ps_tr.tile([128, D], F32R, tag="tr", name="trc")
                    for m in range(2):
                        nc.tensor.transpose(
                            pst[:, m * 128 : (m + 1) * 128],
                            ctxr[:, :, :].rearrange("p h e -> p (h e)")[
                                :, m * 128 : (m + 1) * 128],
                            identr_sb[:])
                    for m in range(2):
                        nc.scalar.activation(
                            ctxT[:, m, t * 128 : (t + 1) * 128],
                            pst[:, m * 128 : (m + 1) * 128].bitcast(F32), AF.Copy)

                # --- out-projection (transposed) ---
                aoT = work.tile([128, 2, NG], F32, tag="aoT", name="aoT")
                for m in range(2):
                    for c in range(NCH):
                        ps = ps_mm.tile([128, CH], F32, tag="mm", name="mm")
                        for kk in range(2):
                            nc.tensor.matmul(
                                ps[:],
                                wo_sb[kk][:, m * 128 : (m + 1) * 128],
                                ctxT[:, kk, c * CH : (c + 1) * CH],
                                start=(kk == 0), stop=(kk == 1))
                        if use_bo:
                            nc.scalar.activation(
                                aoT[:, m, c * CH : (c + 1) * CH], ps[:], AF.Identity,
                                bias=bo_sb[m][:, b : b + 1])
                        else:
                            nc.scalar.activation(
                                aoT[:, m, c * CH : (c + 1) * CH], ps[:], AF.Copy)

            # --- residual + LN1 (row layout), re-transpose to xT ---
            xT = work.tile([128, 2, NG], F32R, tag="xT", name="xT")
            for t in range(NT):
                r1 = work.tile([128, D], F32, tag="r1", name="r1")
                if Lb > 0:
                    pst = ps_tr.tile([128, D], F32, tag="tr", name="tr")
                    for m in range(2):
                        nc.tensor.transpose(
                            pst[:, m * 128 : (m + 1) * 128],
                            aoT[:, m, t * 128 : (t + 1) * 128], ident_sb[:])
                    nc.vector.tensor_add(r1[:], pst[:], hgr_sb[t][:])
                else:
                    nc.vector.tensor_copy(out=r1[:], in_=hgr_sb[t][:])

                stats = work.tile([128, 6], F32, tag="stats", name="stats")
                mv = work.tile([128, 2], F32, tag="mv", name="mv")
                nc.vector.bn_stats(out=stats[:], in_=r1[:])
                nc.vector.bn_aggr(out=mv[:], in_=stats[:])
                nc.scalar.activation(mv[:, 1:2], mv[:, 1:2], AF.Sqrt,
                                     bias=eps_sb[:, 0:1])
                nc.vector.reciprocal(out=mv[:, 1:2], in_=mv[:, 1:2])
                # xr = xhat (unit-affine LN); g1/b1_ln are folded into the FFN
                # weights on host, and applied per-partition in T-layout for
                # the residual below when nontrivial.
                xr = work.tile([128, D], F32, tag="xr", name="xr")
                nc.vector.tensor_scalar(
                    out=xr[:], in0=r1[:], scalar1=mv[:, 0:1], scalar2=mv[:, 1:2],
                    op0=mybir.AluOpType.subtract, op1=mybir.AluOpType.mult)
                pst2 = ps_tr.tile([128, D], F32, tag="tr", name="tr")
                for m in range(2):
                    nc.tensor.transpose(
                        pst2[:, m * 128 : (m + 1) * 128],
                        xr[:, m * 128 : (m + 1) * 128], ident_sb[:])
                for m in range(2):
                    nc.scalar.activation(
                        xT[:, m, t * 128 : (t + 1) * 128],
                        pst2[:, m * 128 : (m + 1) * 128], AF.Copy)

            # --- FFN1 + exact gelu ---
            h1g = h1pool.tile([128, 8, NG], F32R, tag="h1g", name="h1g")
            for m in range(8):
                for c in range(NCH):
                    ps = ps_mm.tile([128, CH], F32, tag="mm", name="mm")
                    for kk in range(2):
                        nc.tensor.matmul(
                            ps[:],
                            w1_sb[kk][:, m * 128 : (m + 1) * 128],
                            xT[:, kk, c * CH : (c + 1) * CH],
                            start=(kk == 0), stop=(kk == 1))
                    if use_b1:
                        nc.scalar.activation(h1g[:, m, c * CH : (c + 1) * CH],
                                             ps[:], AF.Gelu,
                                             bias=b1_sb[m][:, 0:1])
                    else:
                        nc.scalar.activation(h1g[:, m, c * CH : (c + 1) * CH],
                                             ps[:], AF.Gelu)

            # --- FFN2 + residual -> yT ---
            # residual adds x_ln = xhat*g1 + b1_ln; per-partition affine in
            # T-layout when the ln1 affine is nontrivial, else xT directly.
            if use_g1 or use_b1ln:
                xres = work.tile([128, 2, NG], F32, tag="xres", name="xres")
                for m in range(2):
                    nc.vector.tensor_scalar(
                        out=xres[:, m, :], in0=xT[:, m, :].bitcast(F32),
                        scalar1=ln1_sb[m][:, 0:1], scalar2=ln1_sb[m][:, 1:2],
                        op0=mybir.AluOpType.mult, op1=mybir.AluOpType.add)
            else:
                xres = xT
            yT = work.tile([128, 2, NG], F32, tag="yT", name="yT")
            for m in range(2):
                for c in range(NCH):
                    ps = ps_mm.tile([128, CH], F32, tag="mm", name="mm")
                    for kk in range(8):
                        nc.tensor.matmul(
                            ps[:],
                            w2_sb[kk][:, m * 128 : (m + 1) * 128],
                            h1g[:, kk, c * CH : (c + 1) * CH],
                            start=(kk == 0), stop=(kk == 7))
                    if use_b2:
                        nc.vector.scalar_tensor_tensor(
                            out=yT[:, m, c * CH : (c + 1) * CH], in0=ps[:],
                            scalar=b2_sb[m][:, 0:1],
                            in1=xres[:, m, c * CH : (c + 1) * CH].bitcast(F32),
                            op0=mybir.AluOpType.add, op1=mybir.AluOpType.add)
                    else:
                        nc.vector.tensor_add(
                            yT[:, m, c * CH : (c + 1) * CH], ps[:],
                            xres[:, m, c * CH : (c + 1) * CH].bitcast(F32))

            # --- LN2 (row layout) + store ---
            for t in range(NT):
                psy = ps_tr.tile([128, D], F32, tag="tr", name="tr")
                for m in range(2):
                    nc.tensor.transpose(
                        psy[:, m * 128 : (m + 1) * 128],
                        yT[:, m, t * 128 : (t + 1) * 128], ident_sb[:])
                yr = work.tile([128, D], F32, tag="yr", name="yr")
                nc.scalar.activation(yr[:], psy[:], AF.Copy)
                stats = work.tile([128, 6], F32, tag="stats", name="stats")
                mv = work.tile([128, 2], F32, tag="mv", name="mv")
                nc.vector.bn_stats(out=stats[:], in_=yr[:])
                nc.vector.bn_aggr(out=mv[:], in_=stats[:])
                nc.scalar.activation(mv[:, 1:2], mv[:, 1:2], AF.Sqrt,
                                     bias=eps_sb[:, 0:1])
                nc.vector.reciprocal(out=mv[:, 1:2], in_=mv[:, 1:2])
                orow = work.tile([128, D], F32, tag="orow", name="orow")
                nc.vector.tensor_scalar(
                    out=orow[:], in0=yr[:], scalar1=mv[:, 0:1], scalar2=mv[:, 1:2],
                    op0=mybir.AluOpType.subtract, op1=mybir.AluOpType.mult)
                if use_g2:
                    nc.vector.tensor_mul(orow[:], orow[:], gbr_sb[:, 2, :])
                if use_b2ln:
                    nc.vector.tensor_add(orow[:], orow[:], gbr_sb[:, 3, :])
                nc.sync.dma_start(out=out[b, t * 128 : (t + 1) * 128, :],
                                  in_=orow[:])

        cstack.close()

    return nc


def kernel(H_genes, perturbation_indices, batch_assignment, batch_size,
           in_proj_w, in_proj_b, out_proj_w, out_proj_b,
           ffn_w1, ffn_b1, ffn_w2, ffn_b2,
           ln1_g, ln1_b, ln2_g, ln2_b):
    Hg = np.ascontiguousarray(np.asarray(H_genes, dtype=np.float32))
    pidx = np.asarray(perturbation_indices).astype(np.int64)
    ba = np.asarray(batch_assignment).astype(np.int64)
    Bs = int(np.asarray(batch_size))
    assert Bs == B, f"kernel hardcodes B=16, got {Bs}"
    assert Hg.shape == (N, D)

    Wq, Wk, Wv = [np.asarray(w, np.float32) for w in np.split(np.asarray(in_proj_w), 3, axis=0)]
    bq, bk, bv = [np.asarray(x, np.float32) for x in np.split(np.asarray(in_proj_b), 3, axis=0)]
    Wo = np.asarray(out_proj_w, np.float32)
    bo = np.asarray(out_proj_b, np.float32)
    W1 = np.asarray(ffn_w1, np.float32)
    b1 = np.asarray(ffn_b1, np.float32)
    W2 = np.asarray(ffn_w2, np.float32)
    b2 = np.asarray(ffn_b2, np.float32)
    g1 = np.asarray(ln1_g, np.float32)
    be1 = np.asarray(ln1_b, np.float32)
    g2 = np.asarray(ln2_g, np.float32)
    be2 = np.asarray(ln2_b, np.float32)

    # ragged batch ranges (batch_assignment is sorted)
    counts = np.bincount(ba, minlength=B).astype(np.int64)
    starts = np.concatenate([[0], np.cumsum(counts)[:-1]]).astype(np.int64)
    has_any = (counts > 0)

    # block/slot decomposition of the sorted p-ranges
    groups = []
    for g in range(NGRP):
        lo, hi = g * GW, (g + 1) * GW
        sl = []
        for b in range(B):
            s, e = int(starts[b]), int(starts[b] + counts[b])
            s2, e2 = max(s, lo), min(e, hi)
            if s2 < e2:
                sl.append((b, s2, e2 - s2))
        groups.append(sl)
    contribs = {b: [] for b in range(B)}
    for g in range(NGRP):
        for s, (b, _, _) in enumerate(groups[g]):
            contribs[b].append((g, s))

    # fold ln1 affine into FFN1 (exact): W1' = W1*g1, b1' = W1@b1_ln + b1
    W1f = W1 * g1[None, :]
    b1f = b1 + W1 @ be1

    Hp = np.ascontiguousarray(Hg[pidx])             # [P, D]
    Hg_pad = np.zeros((NPAD, D), np.float32)
    Hg_pad[:N] = Hg

    m01 = (ba[:, None] == np.arange(16)[None, :]).astype(np.float32)
    m01bd = np.zeros((NGRP, 128, 128), np.float32)
    for g in range(NGRP):
        for h in range(H):
            m01bd[g, h * GW : (h + 1) * GW, h * GW : (h + 1) * GW] = \
                m01[g * GW : (g + 1) * GW, :]
    emptyp = np.tile((~has_any).astype(np.float32), H)[:, None]
    ident = np.eye(128, dtype=np.float32)
    bo_mask = (bo[:, None] * has_any[None, :].astype(np.float32))  # [D, B]
    gb_row = np.stack([g1, be1, g2, be2], axis=0)                  # [4, D]

    flags = (
        bool(np.any(bq != 0)), bool(np.any(bk != 0)), bool(np.any(bv != 0)),
        bool(np.any(bo != 0)), bool(np.any(b1f != 0)), bool(np.any(b2 != 0)),
        bool(np.any(g1 != 1)), bool(np.any(be1 != 0)),
        bool(np.any(g2 != 1)), bool(np.any(be2 != 0)),
    )

    nc = _build_program(counts, groups, contribs, flags)

    common = {
        "hp_t": np.ascontiguousarray(Hp.T),
        "m01bd": m01bd,
        "emptyp": np.ascontiguousarray(emptyp),
        "ident": ident,
        "identr": ident,
        "identb": np.ascontiguousarray(np.tile(np.eye(32, dtype=np.float32), (4, 1))),
        "wq_t": np.ascontiguousarray(Wq.T),
        "wk_t": np.ascontiguousarray(Wk.T),
        "wv_t": np.ascontiguousarray(Wv.T),
        "wo_t": np.ascontiguousarray(Wo.T),
        "w1_t": np.ascontiguousarray(W1f.T),
        "w2_t": np.ascontiguousarray(W2.T),
        "bias_kv": np.ascontiguousarray(np.stack([bk, bv], axis=1)),
        "bq_col": bq[:, None].copy(),
        "bo_mask": np.ascontiguousarray(bo_mask),
        "b1_col": b1f[:, None].copy(),
        "b2_col": b2[:, None].copy(),
        "ln1_col": np.ascontiguousarray(np.stack([g1, be1], axis=1)),
        "gb_row": gb_row,
        "zeros_r": np.zeros((128, max(1, max(len(g) for g in groups)) * D), np.float32),
    }
    in_maps = []
    for c in range(NCORES):
        sl = Hg_pad[c * NG : (c + 1) * NG]
        m = dict(common)
        m["hg_row"] = np.ascontiguousarray(sl)
        m["hg_t"] = np.ascontiguousarray(sl.T)
        in_maps.append(m)

    if os.environ.get("BASS_KERNEL_SIM"):
        from concourse import bass_interp
        # CoreSim lacks a Gelu implementation; shim in exact (erf) gelu for
        # local debugging (HW uses the ACT LUT).
        if not getattr(bass_interp.InstructionExecutor, "_gelu_patched", False):
            from scipy.special import erf
            _orig_act = bass_interp.InstructionExecutor.visit_InstActivation

            def _act(self, instruction, *, reg_snapshot=None):
                if instruction.func == mybir.ActivationFunctionType.Gelu:
                    instruction.func = mybir.ActivationFunctionType.Identity
                    try:
                        import concourse.bass_interp as bi
                        out_ap = instruction.outs[0]
                        r = _orig_act(self, instruction, reg_snapshot=reg_snapshot)
                        view = self.view_ap(out_ap, bi.Direction.READ, instruction,
                                            reg_snapshot=reg_snapshot)
                        x = view.astype(np.float64)
                        view[:] = (0.5 * x * (1.0 + erf(x / np.sqrt(2.0)))).astype(view.dtype)
                        return r
                    finally:
                        instruction.func = mybir.ActivationFunctionType.Gelu
                return _orig_act(self, instruction, reg_snapshot=reg_snapshot)

            bass_interp.InstructionExecutor.visit_InstActivation = _act
            bass_interp.InstructionExecutor._gelu_patched = True
        nsim = int(os.environ.get("BASS_KERNEL_SIM_CORES", "1"))
        simtrace = bool(os.environ.get("BASS_KERNEL_SIMTRACE"))
        sim = bass_interp.MultiCoreSim(nc, nsim, trace=simtrace)
        for c in range(nsim):
            for k, v in in_maps[c].items():
                sim.cores[c].tensor(k)[:] = v
        sim.simulate()
        print(f"SIM predicted time: {sim.cores[0].time} ns")
        full = np.zeros((B, NPAD, D), np.float32)
        for c in range(nsim):
            full[:, c * NG : (c + 1) * NG, :] = (
                np.array(sim.cores[c].mem_tensor("out")).reshape(B, NG, D))
        return full[:, :N, :]

    from concourse.bass_utils import run_bass_kernel_spmd
    _split_waits(nc)
    trace = bool(os.environ.get("BASS_KERNEL_TRACE"))
    res = run_bass_kernel_spmd(nc, in_maps, core_ids=list(range(NCORES)),
                               trace=trace)
    if trace and res.exec_time_ns is not None:
        print(f"HW exec time: {res.exec_time_ns} ns")
        if res.instructions_and_trace:
            print("trace:", res.instructions_and_trace[1])

    full = np.zeros((B, NPAD, D), np.float32)
    for c in range(NCORES):
        full[:, c * NG : (c + 1) * NG, :] = res.results[c]["out"]
    return full[:, :N, :]



# revision 19
# speedup vs baseline: 1.7816x; 1.7816x over previous
"""Trainium2 Bass kernel for nn_EquivariantPerturbationTransform.

Reference (N=6000 genes, D=256, H=8 heads, P=128 perturbations, B=16):
  q = H @ Wq.T ; k,v from gathered perturbation rows
  scores[h,n,p] shared across batches; per-batch mask over p (ragged)
  attn_out[b] = softmax-masked attention -> out proj (zeroed for empty b)
  x = LN1(H + attn_out); out = LN2(x + gelu(x@W1.T)@W2.T)

v2 strategy (sequence-parallel over 8 cores, 768 query rows/core):
  - Wo is folded into the values on the host: vo[(h,p),:] = v[p,h] @ Wo_h.T,
    so attention context IS the projected attn_out -- the per-batch [N,D]@[D,D]
    out-projection disappears and the attention matmul lands in row layout.
  - Normalized masked attention weights built per (batch, 16-block):
    a replication matmul (0/1 stationary R_bg) broadcasts reciprocal softmax
    denominators from [(h,b),n] packed layout into [(h,p16),n] block layout
    (zeroing masked rows), then one gpsimd multiply gives Ehat = E * rep.
  - LN1 in row layout straight from the attention PSUM; x transposed (bf16,
    1 cyc/row) to T layout for FFN1; FFN2 in T layout; the PSUM->SBUF evac of
    y adds the x residual for free (gpsimd tensor_add); y transposed back for
    LN2 + store.
  - rsqrt for the LNs via a single DVE tensor_scalar (add eps, pow -0.5) --
    keeps Sqrt off the ACT engine so the activation table never leaves
    gelu_and_others (Gelu/Identity/Copy) after the phase-A Exp.
  - bf16 matmuls everywhere except q/k scores (fp32r: exp amplifies score
    rounding); fp32 PSUM accumulation; fp32 H residual into LN1.
  - Engine split: PE matmuls/transposes; DVE LN stats+applies+recip; gpsimd
    Ehat muls, r1 adds, xT copies, y evac+residual; ACT exp+gelu only.
"""

import os
import sys

sys.path.insert(0, "/opt/trn_rl_repo")

import numpy as np

import concourse.bass as bass
from concourse import mybir
from concourse.tile import TileContext

F32 = mybir.dt.float32
F32R = mybir.dt.float32r
BF16 = mybir.dt.bfloat16
AF = mybir.ActivationFunctionType
ALU = mybir.AluOpType

N, D, H, P, B = 6000, 256, 8, 128, 16
DH = D // H  # 32
NCORES = 8
NPAD = 6144
NG = NPAD // NCORES  # 768 rows per core
NT = NG // 128       # 6 row tiles
NCH = 2              # 384-wide chunks
CH = NG // NCH
EPS = 1e-5
GW = 16              # perturbation block width
NGRP = P // GW       # 8 blocks
F1 = 4 * D           # 1024


def _split_waits(nc, max_waits=1):
    """The neuronxcc/walrus build here rejects >1 sync-wait per instruction;
    hoist excess waits onto same-engine NoOps (semantically identical)."""
    n_split = 0
    for f in nc.m.functions:
        for bb in f.blocks:
            new_list = []
            for ins in bb.instructions:
                si = getattr(ins, "sync_info", None)
                if si is not None and si.on_wait and len(si.on_wait) > max_waits:
                    waits = list(si.on_wait)
                    excess, keep = waits[:-max_waits], waits[-max_waits:]
                    for i in range(0, len(excess), max_waits):
                        chunk = excess[i : i + max_waits]
                        nop = mybir.InstNoOp(name=f"{ins.name}-ws{i}", ins=[], outs=[])
                        nop.engine = ins.engine
                        nop.sync_info = mybir.SyncInfo(on_wait=chunk, on_update=[])
                        new_list.append(nop)
                        n_split += 1
                    si.on_wait = keep
                new_list.append(ins)
            bb.instructions = new_list
    return n_split


def _build_program(counts, blocks, nrep, flags):
    """blocks[b] = list of (g, rep_idx) block/replication-matrix pairs."""
    (use_bq, use_b1, use_b2, use_g1, use_b1ln, use_g2, use_b2ln) = flags
    nc = bass.Bass()

    # ---- DRAM parameters -------------------------------------------------
    hgb_row = nc.declare_dram_parameter("hgb_row", [NG, D], F32R, isOutput=False)
    hg_row = nc.declare_dram_parameter("hg_row", [NG, D], F32R, isOutput=False)
    hg_t = nc.declare_dram_parameter("hg_t", [D, NG], F32R, isOutput=False)
    kt = nc.declare_dram_parameter("kt", [D, P], F32R, isOutput=False)
    wq_t = nc.declare_dram_parameter("wq_t", [D, D], F32R, isOutput=False)
    bq_col = nc.declare_dram_parameter("bq_col", [D, 1], F32, isOutput=False)
    vo = nc.declare_dram_parameter("vo", [NGRP, 128, D], BF16, isOutput=False)
    m01bd = nc.declare_dram_parameter("m01bd", [NGRP, 128, 128], BF16, isOutput=False)
    repm = nc.declare_dram_parameter("repm", [max(nrep, 1), 128, 128], BF16, isOutput=False)
    identb = nc.declare_dram_parameter("identb", [128, 128], BF16, isOutput=False)
    identf = nc.declare_dram_parameter("identf", [128, 128], F32, isOutput=False)
    identr = nc.declare_dram_parameter("identr", [128, 128], F32R, isOutput=False)
    emptyp = nc.declare_dram_parameter("emptyp", [128, 1], F32, isOutput=False)
    w1_t = nc.declare_dram_parameter("w1_t", [D, F1], BF16, isOutput=False)
    w2_t = nc.declare_dram_parameter("w2_t", [F1, D], BF16, isOutput=False)
    b1_col = nc.declare_dram_parameter("b1_col", [F1, 1], F32, isOutput=False)
    ln1_col = nc.declare_dram_parameter("ln1_col", [D, 2], F32, isOutput=False)
    gb_row = nc.declare_dram_parameter("gb_row", [4, D], F32, isOutput=False)
    out = nc.declare_dram_parameter("out", [B, NG, D], F32, isOutput=True)

    s_attn = 1.0 / float(np.sqrt(DH))
    any_empty = any(int(c) == 0 for c in counts)
    jobs = [b for b in range(B) if int(counts[b]) > 0]

    with TileContext(nc) as tc, nc.allow_low_precision(
            reason="bf16 matmul inputs; tolerance budget is 2e-2 of max"):
        import contextlib

        cstack = contextlib.ExitStack()
        consts = cstack.enter_context(tc.tile_pool(name="consts", bufs=1))

        # ---- persistent constants -------------------------------------
        def load_w(name, ap, rows, cols, dt):
            tiles = []
            for k in range(rows // 128):
                tl = consts.tile([128, cols], dt, tag=f"{name}{k}", name=f"{name}{k}")
                nc.sync.dma_start(out=tl[:], in_=ap[k * 128 : (k + 1) * 128, :])
                tiles.append(tl)
            return tiles

        hgb_sb = load_w("hgb", hgb_row, NG, D, F32R)  # row tiles (residual)
        hgt_sb = load_w("hgt", hg_t, D, NG, F32R)
        kt_sb = load_w("kt", kt, D, P, F32R)
        wq_sb = load_w("wq", wq_t, D, D, F32R)
        w1_sb = load_w("w1", w1_t, D, F1, BF16)
        w2_sb = load_w("w2", w2_t, F1, D, BF16)
        hg_sb = load_w("hg", hg_row, NG, D, F32R) if any_empty else None

        vo_sb = []
        for g in range(NGRP):
            tl = consts.tile([128, D], BF16, tag=f"vo{g}", name=f"vo{g}")
            nc.sync.dma_start(out=tl[:], in_=vo[g, :, :])
            vo_sb.append(tl)
        m01_sb = []
        for g in range(NGRP):
            tl = consts.tile([128, 128], BF16, tag=f"m01{g}", name=f"m01{g}")
            nc.sync.dma_start(out=tl[:], in_=m01bd[g, :, :])
            m01_sb.append(tl)
        rep_sb = []
        for i in range(nrep):
            tl = consts.tile([128, 128], BF16, tag=f"rep{i}", name=f"rep{i}")
            nc.sync.dma_start(out=tl[:], in_=repm[i, :, :])
            rep_sb.append(tl)
        identb_sb = consts.tile([128, 128], BF16, tag="identb", name="identb")
        nc.sync.dma_start(out=identb_sb[:], in_=identb[:, :])
        identf_sb = consts.tile([128, 128], F32, tag="identf", name="identf")
        nc.sync.dma_start(out=identf_sb[:], in_=identf[:, :])
        identr_sb = consts.tile([128, 128], F32R, tag="identr", name="identr")
        nc.sync.dma_start(out=identr_sb[:], in_=identr[:, :])
        empty_sb = consts.tile([128, 1], F32, tag="empty", name="empty")
        nc.sync.dma_start(out=empty_sb[:], in_=emptyp[:, :])
        b1_sb = load_w("b1c", b1_col, F1, 1, F32) if use_b1 else None
        bq_sb = load_w("bqc", bq_col, D, 1, F32) if use_bq else None
        ln1_sb = load_w("ln1c", ln1_col, D, 2, F32) if (use_g1 or use_b1ln) else None
        gbr_sb = None
        if use_g2 or use_b2ln or use_b2:
            gbr_sb = consts.tile([128, 4, D], F32, tag="gbr", name="gbr")
            nc.gpsimd.dma_start(out=gbr_sb[:], in_=gb_row[:, :].to_broadcast((128, 4, D)))

        # persistent activations
        qT_sb = [consts.tile([128, NG], F32R, tag=f"qT{i}", name=f"qT{i}")
                 for i in range(2)]
        Et = consts.tile([128, H, NG], BF16, tag="Et", name="Et")
        Eg = [consts.tile([128, NG], BF16, tag=f"Eg{g}", name=f"Eg{g}")
              for g in range(NGRP)]
        den_sb = consts.tile([128, NG], F32, tag="den", name="den")
        denr_sb = consts.tile([128, NG], BF16, tag="denr", name="denr")

        # ================= Phase A: shared projections ==================
        with tc.tile_pool(name="psA", bufs=2, space="PSUM") as psA:
            # qT [D, NG]
            for m in range(2):
                for c in range(NCH):
                    ps = psA.tile([128, CH], F32, tag="ps", name="ps")
                    for kk in range(2):
                        nc.tensor.matmul(
                            ps[:],
                            wq_sb[kk][:, m * 128 : (m + 1) * 128],
                            hgt_sb[kk][:, c * CH : (c + 1) * CH],
                            start=(kk == 0), stop=(kk == 1))
                    if use_bq:
                        nc.scalar.activation(
                            qT_sb[m][:, c * CH : (c + 1) * CH], ps[:], AF.Identity,
                            bias=bq_sb[m][:, 0:1])
                    else:
                        nc.scalar.activation(
                            qT_sb[m][:, c * CH : (c + 1) * CH], ps[:], AF.Copy)

            # E^T per head: Et[p, h, n] = exp(s * k_h q_h^T)
            for h in range(H):
                ktt = kt_sb[h // 4]
                for c in range(NCH):
                    ps = psA.tile([128, CH], F32, tag="ps", name="ps")
                    nc.tensor.matmul(
                        ps[:],
                        ktt[(h % 4) * DH : (h % 4 + 1) * DH, :],
                        qT_sb[h // 4][(h % 4) * DH : (h % 4 + 1) * DH,
                                      c * CH : (c + 1) * CH],
                        start=True, stop=True,
                        tile_position=((h % 4) * DH, 0))
                    nc.scalar.activation(Et[:, h, c * CH : (c + 1) * CH],
                                         ps[:], AF.Exp, scale=s_attn)

            # regroup E into per-block layout [(h,p16), n]
            for g in range(NGRP):
                for h in range(H):
                    nc.sync.dma_start(
                        out=Eg[g][h * GW : (h + 1) * GW, :],
                        in_=Et[g * GW : (g + 1) * GW, h, :])

            # softmax denominators, packed [(h,b), n]; +1 on empty batches
            for c in range(NCH):
                psd = psA.tile([128, CH], F32, tag="psd", name="psd")
                for g in range(NGRP):
                    nc.tensor.matmul(
                        psd[:], m01_sb[g][:], Eg[g][:, c * CH : (c + 1) * CH],
                        start=(g == 0), stop=(g == NGRP - 1))
                nc.vector.tensor_scalar(
                    out=den_sb[:, c * CH : (c + 1) * CH], in0=psd[:],
                    scalar1=empty_sb[:, 0:1], scalar2=None, op0=ALU.add)
            nc.vector.reciprocal(out=denr_sb[:], in_=den_sb[:])

        # ================= Phase B: per-batch back half =================
        # Software-pipelined: front half (attention + LN1 + xT) of batch j
        # is emitted together with the back half (FFN + LN2 + store) of
        # batch j-1, so the PE never waits on the LN1 DVE/ACT chain.
        work = cstack.enter_context(tc.tile_pool(name="work", bufs=2))
        ehpool = cstack.enter_context(tc.tile_pool(name="ehp", bufs=2))
        ps_ao = cstack.enter_context(tc.tile_pool(name="ps_ao", bufs=3, space="PSUM"))
        ps_tp = cstack.enter_context(tc.tile_pool(name="ps_tp", bufs=3, space="PSUM"))
        ps_ff = cstack.enter_context(tc.tile_pool(name="ps_ff", bufs=2, space="PSUM"))

        RC2, RC1, RC0 = 0.25624194572868153, -1.1105733501887358, 1.8846770479237582

        def rsqrt_group(vv_ap, mu_ap, sc_ap, ng_ap, sA, sB):
            """sc = rsqrt(vv), ng = -mu*sc on a [128, 3] strided slice.
            Quadratic seed + 2 Newton steps; DVE-valid ops only."""
            nc.vector.tensor_scalar(out=sA, in0=vv_ap, scalar1=RC2,
                                    scalar2=RC1, op0=ALU.mult, op1=ALU.add)
            nc.vector.tensor_tensor(out=sA, in0=sA, in1=vv_ap, op=ALU.mult)
            nc.vector.tensor_scalar(out=sA, in0=sA, scalar1=RC0,
                                    scalar2=None, op0=ALU.add)
            for _ in range(2):
                nc.vector.tensor_tensor(out=sB, in0=sA, in1=sA, op=ALU.mult)
                nc.vector.tensor_tensor(out=sB, in0=sB, in1=vv_ap, op=ALU.mult)
                nc.vector.tensor_scalar(out=sB, in0=sB, scalar1=-0.5,
                                        scalar2=1.5, op0=ALU.mult, op1=ALU.add)
                nc.vector.tensor_tensor(out=sA, in0=sA, in1=sB, op=ALU.mult)
            nc.vector.tensor_copy(out=sc_ap, in_=sA)
            nc.vector.scalar_tensor_tensor(
                out=ng_ap, in0=mu_ap, scalar=-1.0, in1=sc_ap,
                op0=ALU.mult, op1=ALU.mult)

        def do_front(b, bl):
            """Attention + residual + LN1 + x transpose. Returns tiles for
            the back half. b None => empty-batch base job (no attention)."""
            nb = len(bl)
            eh = []
            if nb:
                for i, (g, ri) in enumerate(bl):
                    tl = ehpool.tile([128, NG], BF16, tag=f"eh{i}", name=f"eh{i}")
                    eh.append(tl)
                for c in range(NCH):
                    for i, (g, ri) in enumerate(bl):
                        psr = ps_tp.tile([128, CH], F32, tag="tp", name="rep")
                        nc.tensor.matmul(
                            psr[:], rep_sb[ri][:],
                            denr_sb[:, c * CH : (c + 1) * CH],
                            start=True, stop=True)
                        nc.vector.tensor_mul(
                            eh[i][:, c * CH : (c + 1) * CH],
                            Eg[g][:, c * CH : (c + 1) * CH], psr[:])

            mvb = work.tile([128, NT, 2], F32, tag="mvb", name="mvb")
            sc1 = work.tile([128, NT], F32, tag="sc1", name="sc1")
            ng1 = work.tile([128, NT], F32, tag="ng1", name="ng1")
            stats = work.tile([128, 6], F32, tag="stats", name="stats")
            x_row = work.tile([128, NT, D], BF16, tag="x_row", name="x_row")
            xT = work.tile([128, 2, NG], BF16, tag="xT", name="xT")
            sA = work.tile([128, 3], F32, tag="sA", name="sA")
            sB = work.tile([128, 3], F32, tag="sB", name="sB")
            hrow = hgb_sb if (b is not None) else (hg_sb or hgb_sb)
            for c in range(NCH):
                t0 = c * 3
                pg = []
                for t in range(t0, t0 + 3):
                    psa = ps_ao.tile([128, D], F32, tag="ao", name="ao")
                    if nb:
                        for i, (g, ri) in enumerate(bl):
                            nc.tensor.matmul(
                                psa[:], eh[i][:, t * 128 : (t + 1) * 128],
                                vo_sb[g][:], start=(i == 0), stop=False)
                    # exact H (+bo) residual accumulated on the PE (fp32r)
                    nc.tensor.matmul(psa[:], identr_sb[:], hrow[t][:],
                                     start=(nb == 0), stop=True)
                    nc.vector.bn_stats(out=stats[:], in_=psa[:])
                    nc.vector.bn_aggr(out=mvb[:, t, :], in_=stats[:])
                    pg.append(psa)
                rsqrt_group(mvb[:, t0 : t0 + 3, 1], mvb[:, t0 : t0 + 3, 0],
                            sc1[:, t0 : t0 + 3], ng1[:, t0 : t0 + 3],
                            sA[:], sB[:])
                for t in range(t0, t0 + 3):
                    nc.scalar.activation(
                        x_row[:, t, :], pg[t - t0][:], AF.Identity,
                        bias=ng1[:, t : t + 1], scale=sc1[:, t : t + 1])
                    psx = ps_tp.tile([128, D], BF16, tag="tp", name="xp")
                    for k in range(2):
                        nc.tensor.transpose(
                            psx[:, k * 128 : (k + 1) * 128],
                            x_row[:, t, k * 128 : (k + 1) * 128],
                            identb_sb[:])
                    for k in range(2):
                        nc.vector.tensor_copy(
                            out=xT[:, k, t * 128 : (t + 1) * 128],
                            in_=psx[:, k * 128 : (k + 1) * 128])

            # residual operand in T layout (affine of xhat when LN1 nontrivial)
            if use_g1 or use_b1ln:
                xres = work.tile([128, 2, NG], BF16, tag="xres", name="xres")
                for k in range(2):
                    nc.gpsimd.tensor_scalar(
                        out=xres[:, k, :], in0=xT[:, k, :],
                        scalar1=ln1_sb[k][:, 0:1], scalar2=ln1_sb[k][:, 1:2],
                        op0=ALU.mult, op1=ALU.add)
            else:
                xres = xT
            return xT, xres

        def do_back(b, xT, xres):
            # --- FFN1 + gelu (768-wide moving) ---
            h1g = work.tile([128, 8, NG], BF16, tag="h1g", name="h1g")
            for c in range(NCH):
                for m in range(8):
                    ps = ps_ff.tile([128, CH], F32, tag="ff", name="f1")
                    for kk in range(2):
                        nc.tensor.matmul(
                            ps[:], w1_sb[kk][:, m * 128 : (m + 1) * 128],
                            xT[:, kk, c * CH : (c + 1) * CH],
                            start=(kk == 0), stop=(kk == 1))
                    if use_b1:
                        nc.scalar.activation(h1g[:, m, c * CH : (c + 1) * CH],
                                             ps[:], AF.Gelu, bias=b1_sb[m][:, 0:1])
                    else:
                        nc.scalar.activation(h1g[:, m, c * CH : (c + 1) * CH],
                                             ps[:], AF.Gelu)

            # --- FFN2 (T layout) + x residual on the PE; DMA evac ---
            yT = work.tile([128, 2, NG], BF16, tag="yT", name="yT")
            for c in range(NCH):
                for m in range(2):
                    ps = ps_ff.tile([128, CH], F32, tag="ff", name="y")
                    for kk in range(8):
                        nc.tensor.matmul(
                            ps[:], w2_sb[kk][:, m * 128 : (m + 1) * 128],
                            h1g[:, kk, c * CH : (c + 1) * CH],
                            start=(kk == 0), stop=False)
                    nc.tensor.matmul(
                        ps[:], identb_sb[:], xres[:, m, c * CH : (c + 1) * CH],
                        start=False, stop=True)
                    nc.scalar.activation(yT[:, m, c * CH : (c + 1) * CH],
                                         ps[:], AF.Copy)

            # --- transpose back, LN2, store ---
            mv2 = work.tile([128, NT, 2], F32, tag="mv2", name="mv2")
            sc2 = work.tile([128, NT], F32, tag="sc2", name="sc2")
            ng2 = work.tile([128, NT], F32, tag="ng2", name="ng2")
            st2 = work.tile([128, 6], F32, tag="st2", name="st2")
            sA2 = work.tile([128, 3], F32, tag="sA2", name="sA2")
            sB2 = work.tile([128, 3], F32, tag="sB2", name="sB2")
            for c in range(NCH):
                t0 = c * 3
                pg = []
                for t in range(t0, t0 + 3):
                    psy = ps_tp.tile([128, D], BF16, tag="tp", name="yt")
                    for k in range(2):
                        nc.tensor.transpose(
                            psy[:, k * 128 : (k + 1) * 128],
                            yT[:, k, t * 128 : (t + 1) * 128], identb_sb[:])
                    if use_b2:
                        nc.vector.tensor_add(psy[:], psy[:], gbr_sb[:, 0, :])
                    nc.vector.bn_stats(out=st2[:], in_=psy[:])
                    nc.vector.bn_aggr(out=mv2[:, t, :], in_=st2[:])
                    pg.append(psy)
                rsqrt_group(mv2[:, t0 : t0 + 3, 1], mv2[:, t0 : t0 + 3, 0],
                            sc2[:, t0 : t0 + 3], ng2[:, t0 : t0 + 3],
                            sA2[:], sB2[:])
                for t in range(t0, t0 + 3):
                    psy = pg[t - t0]
                    orow = work.tile([128, D], F32, tag=f"or{t % 3}", name="orow")
                    nc.scalar.activation(
                        orow[:], psy[:], AF.Identity,
                        bias=ng2[:, t : t + 1], scale=sc2[:, t : t + 1])
                    if use_g2:
                        nc.vector.tensor_mul(orow[:], orow[:], gbr_sb[:, 2, :])
                    if use_b2ln:
                        nc.vector.tensor_add(orow[:], orow[:], gbr_sb[:, 3, :])
                    if b is not None:
                        nc.sync.dma_start(
                            out=out[b, t * 128 : (t + 1) * 128, :], in_=orow[:])
                    else:
                        for be in range(B):
                            if int(counts[be]) == 0:
                                nc.sync.dma_start(
                                    out=out[be, t * 128 : (t + 1) * 128, :],
                                    in_=orow[:])

        alljobs = [(b, blocks[b]) for b in jobs]
        if any_empty:
            alljobs.append((None, []))
        pending = None
        for b, bl in alljobs:
            fr = do_front(b, bl)
            if pending is not None:
                do_back(pending[0], *pending[1])
            pending = (b, fr)
        do_back(pending[0], *pending[1])

        cstack.close()

    return nc


def kernel(H_genes, perturbation_indices, batch_assignment, batch_size,
           in_proj_w, in_proj_b, out_proj_w, out_proj_b,
           ffn_w1, ffn_b1, ffn_w2, ffn_b2,
           ln1_g, ln1_b, ln2_g, ln2_b):
    import ml_dtypes
    bf16 = ml_dtypes.bfloat16

    Hg = np.ascontiguousarray(np.asarray(H_genes, dtype=np.float32))
    pidx = np.asarray(perturbation_indices).astype(np.int64)
    ba = np.asarray(batch_assignment).astype(np.int64)
    Bs = int(np.asarray(batch_size))
    assert Bs == B, f"kernel hardcodes B=16, got {Bs}"
    assert Hg.shape == (N, D)

    Wq, Wk, Wv = [np.asarray(w, np.float32) for w in np.split(np.asarray(in_proj_w), 3, axis=0)]
    bq, bk, bv = [np.asarray(x, np.float32) for x in np.split(np.asarray(in_proj_b), 3, axis=0)]
    Wo = np.asarray(out_proj_w, np.float32)
    bo = np.asarray(out_proj_b, np.float32)
    W1 = np.asarray(ffn_w1, np.float32)
    b1 = np.asarray(ffn_b1, np.float32)
    W2 = np.asarray(ffn_w2, np.float32)
    b2 = np.asarray(ffn_b2, np.float32)
    g1 = np.asarray(ln1_g, np.float32)
    be1 = np.asarray(ln1_b, np.float32)
    g2 = np.asarray(ln2_g, np.float32)
    be2 = np.asarray(ln2_b, np.float32)

    counts = np.bincount(ba, minlength=B).astype(np.int64)
    has_any = counts > 0

    # host-side small projections: k and Wo-folded values
    Hp = Hg[pidx]                                   # [P, D]
    k = Hp @ Wk.T + bk[None, :]                     # [P, D]
    v = Hp @ Wv.T + bv[None, :]                     # [P, D]
    # vo[(h,p),:] = v[p, h-slice] @ Wo[:, h-slice].T  (full attn_out proj)
    vo = np.zeros((NGRP, 128, D), np.float32)
    for g in range(NGRP):
        for h in range(H):
            vh = v[g * GW : (g + 1) * GW, h * DH : (h + 1) * DH]   # [16, 32]
            vo[g, h * GW : (h + 1) * GW, :] = vh @ Wo[:, h * DH : (h + 1) * DH].T

    # block-diagonal per-head mask matrices for denominators
    m01 = (ba[:, None] == np.arange(B)[None, :]).astype(np.float32)  # [P, B]
    m01bd = np.zeros((NGRP, 128, 128), np.float32)
    for g in range(NGRP):
        for h in range(H):
            m01bd[g, h * GW : (h + 1) * GW, h * GW : (h + 1) * GW] = \
                m01[g * GW : (g + 1) * GW, :]

    # replication matrices: rep[(h,b'),(h,p16)] = 1{b'==b and ba[p]==b}
    blocks = {b: [] for b in range(B)}
    rep_mats = []
    for b in range(B):
        if counts[b] == 0:
            continue
        for g in range(NGRP):
            psel = np.where(ba[g * GW : (g + 1) * GW] == b)[0]
            if len(psel) == 0:
                continue
            R = np.zeros((128, 128), np.float32)
            for h in range(H):
                for p16 in psel:
                    R[h * GW + b, h * GW + p16] = 1.0
            blocks[b].append((g, len(rep_mats)))
            rep_mats.append(R)
    nrep = len(rep_mats)

    # fold ln1 gain into FFN1 (exact): W1' = W1*g1, b1' = W1@b1_ln + b1
    W1f = W1 * g1[None, :]
    b1f = b1 + W1 @ be1

    Hg_pad = np.zeros((NPAD, D), np.float32)
    Hg_pad[:N] = Hg
    emptyp = np.tile((~has_any).astype(np.float32), H)[:, None]  # [(h,b),1]

    flags = (
        bool(np.any(bq != 0)), bool(np.any(b1f != 0)), bool(np.any(b2 != 0)),
        bool(np.any(g1 != 1)), bool(np.any(be1 != 0)),
        bool(np.any(g2 != 1)), bool(np.any(be2 != 0)),
    )
    use_bo = bool(np.any(bo != 0))

    nc = _build_program(counts, blocks, nrep, flags)

    common = {
        "kt": np.ascontiguousarray(k.T),
        "wq_t": np.ascontiguousarray(Wq.T),
        "bq_col": bq[:, None].copy(),
        "vo": vo.astype(bf16),
        "m01bd": m01bd.astype(bf16),
        "repm": (np.stack(rep_mats) if nrep else np.zeros((1, 128, 128), np.float32)).astype(bf16),
        "identb": np.eye(128, dtype=np.float32).astype(bf16),
        "identf": np.eye(128, dtype=np.float32),
        "identr": np.eye(128, dtype=np.float32),
        "emptyp": np.ascontiguousarray(emptyp),
        "w1_t": np.ascontiguousarray(W1f.T).astype(bf16),
        "w2_t": np.ascontiguousarray(W2.T).astype(bf16),
        "b1_col": b1f[:, None].copy(),
        "ln1_col": np.ascontiguousarray(np.stack([g1, be1], axis=1)),
        "gb_row": np.stack([b2, be1, g2, be2], axis=0),
    }
    in_maps = []
    for c in range(NCORES):
        sl = Hg_pad[c * NG : (c + 1) * NG]
        m = dict(common)
        m["hg_row"] = np.ascontiguousarray(sl)
        m["hgb_row"] = np.ascontiguousarray(sl + bo[None, :]) if use_bo else m["hg_row"]
        m["hg_t"] = np.ascontiguousarray(sl.T)
        in_maps.append(m)

    if os.environ.get("BASS_KERNEL_SIM"):
        from concourse import bass_interp
        # CoreSim lacks a Gelu LUT; shim exact (erf) gelu for local debugging.
        if not getattr(bass_interp.InstructionExecutor, "_gelu_patched", False):
            from scipy.special import erf
            _orig_act = bass_interp.InstructionExecutor.visit_InstActivation

            def _act(self, instruction, *, reg_snapshot=None):
                if instruction.func == mybir.ActivationFunctionType.Gelu:
                    instruction.func = mybir.ActivationFunctionType.Identity
                    try:
                        import concourse.bass_interp as bi
                        out_ap = instruction.outs[0]
                        r = _orig_act(self, instruction, reg_snapshot=reg_snapshot)
                        view = self.view_ap(out_ap, bi.Direction.READ, instruction,
                                            reg_snapshot=reg_snapshot)
                        x = view.astype(np.float64)
                        view[:] = (0.5 * x * (1.0 + erf(x / np.sqrt(2.0)))).astype(view.dtype)
                        return r
                    finally:
                        instruction.func = mybir.ActivationFunctionType.Gelu
                return _orig_act(self, instruction, reg_snapshot=reg_snapshot)

            bass_interp.InstructionExecutor.visit_InstActivation = _act
            bass_interp.InstructionExecutor._gelu_patched = True
        nsim = int(os.environ.get("BASS_KERNEL_SIM_CORES", "1"))
        simtrace = bool(os.environ.get("BASS_KERNEL_SIMTRACE"))
        sim = bass_interp.MultiCoreSim(nc, nsim, trace=simtrace)
        for c in range(nsim):
            for kk, vv in in_maps[c].items():
                sim.cores[c].tensor(kk)[:] = vv
        sim.simulate()
        print(f"SIM predicted time: {sim.cores[0].time} ns")
        full = np.zeros((B, NPAD, D), np.float32)
        for c in range(nsim):
            full[:, c * NG : (c + 1) * NG, :] = (
                np.array(sim.cores[c].mem_tensor("out")).reshape(B, NG, D))
        return full[:, :N, :]

    from concourse.bass_utils import run_bass_kernel_spmd
    _split_waits(nc)
    trace = bool(os.environ.get("BASS_KERNEL_TRACE"))
    res = run_bass_kernel_spmd(nc, in_maps, core_ids=list(range(NCORES)),
                               trace=trace)
    if trace and res.exec_time_ns is not None:
        print(f"HW exec time: {res.exec_time_ns} ns")
        if res.instructions_and_trace:
            print("trace:", res.instructions_and_trace[1])

    full = np.zeros((B, NPAD, D), np.float32)
    for c in range(NCORES):
        full[:, c * NG : (c + 1) * NG, :] = res.results[c]["out"]
    return full[:, :N, :]


# revision 21
# speedup vs baseline: 1.9373x; 1.0874x over previous
"""Trainium2 Bass kernel for nn_EquivariantPerturbationTransform.

Reference (N=6000 genes, D=256, H=8 heads, P=128 perturbations, B=16):
  q = H @ Wq.T ; k,v from gathered perturbation rows
  scores[h,n,p] shared across batches; per-batch mask over p (ragged)
  attn_out[b] = softmax-masked attention -> out proj (zeroed for empty b)
  x = LN1(H + attn_out); out = LN2(x + gelu(x@W1.T)@W2.T)

v2 strategy (sequence-parallel over 8 cores, 768 query rows/core):
  - Wo is folded into the values on the host: vo[(h,p),:] = v[p,h] @ Wo_h.T,
    so attention context IS the projected attn_out -- the per-batch [N,D]@[D,D]
    out-projection disappears and the attention matmul lands in row layout.
  - Normalized masked attention weights built per (batch, 16-block):
    a replication matmul (0/1 stationary R_bg) broadcasts reciprocal softmax
    denominators from [(h,b),n] packed layout into [(h,p16),n] block layout
    (zeroing masked rows), then one gpsimd multiply gives Ehat = E * rep.
  - LN1 in row layout straight from the attention PSUM; x transposed (bf16,
    1 cyc/row) to T layout for FFN1; FFN2 in T layout; the PSUM->SBUF evac of
    y adds the x residual for free (gpsimd tensor_add); y transposed back for
    LN2 + store.
  - rsqrt for the LNs via a single DVE tensor_scalar (add eps, pow -0.5) --
    keeps Sqrt off the ACT engine so the activation table never leaves
    gelu_and_others (Gelu/Identity/Copy) after the phase-A Exp.
  - bf16 matmuls everywhere except q/k scores (fp32r: exp amplifies score
    rounding); fp32 PSUM accumulation; fp32 H residual into LN1.
  - Engine split: PE matmuls/transposes; DVE LN stats+applies+recip; gpsimd
    Ehat muls, r1 adds, xT copies, y evac+residual; ACT exp+gelu only.
"""

import os
import sys

sys.path.insert(0, "/opt/trn_rl_repo")

import numpy as np

import concourse.bass as bass
from concourse import mybir
from concourse.tile import TileContext

F32 = mybir.dt.float32
F32R = mybir.dt.float32r
BF16 = mybir.dt.bfloat16
AF = mybir.ActivationFunctionType
ALU = mybir.AluOpType

N, D, H, P, B = 6000, 256, 8, 128, 16
DH = D // H  # 32
NCORES = 8
NPAD = 6144
NG = NPAD // NCORES  # 768 rows per core
NT = NG // 128       # 6 row tiles
NCH = 2              # 384-wide chunks
CH = NG // NCH
EPS = 1e-5
GW = 16              # perturbation block width
NGRP = P // GW       # 8 blocks
F1 = 4 * D           # 1024


def _split_waits(nc, max_waits=1):
    """The neuronxcc/walrus build here rejects >1 sync-wait per instruction;
    hoist excess waits onto same-engine NoOps (semantically identical)."""
    n_split = 0
    for f in nc.m.functions:
        for bb in f.blocks:
            new_list = []
            for ins in bb.instructions:
                si = getattr(ins, "sync_info", None)
                if si is not None and si.on_wait and len(si.on_wait) > max_waits:
                    waits = list(si.on_wait)
                    excess, keep = waits[:-max_waits], waits[-max_waits:]
                    for i in range(0, len(excess), max_waits):
                        chunk = excess[i : i + max_waits]
                        nop = mybir.InstNoOp(name=f"{ins.name}-ws{i}", ins=[], outs=[])
                        nop.engine = ins.engine
                        nop.sync_info = mybir.SyncInfo(on_wait=chunk, on_update=[])
                        new_list.append(nop)
                        n_split += 1
                    si.on_wait = keep
                new_list.append(ins)
            bb.instructions = new_list
    return n_split


def _build_program(counts, blocks, nrep, flags):
    """blocks[b] = list of (g, rep_idx) block/replication-matrix pairs."""
    (use_bq, use_b1, use_b2, use_g1, use_b1ln, use_g2, use_b2ln) = flags
    nc = bass.Bass()

    # ---- DRAM parameters -------------------------------------------------
    hgb_row = nc.declare_dram_parameter("hgb_row", [NG, D], F32R, isOutput=False)
    hg_row = nc.declare_dram_parameter("hg_row", [NG, D], F32R, isOutput=False)
    hg_t = nc.declare_dram_parameter("hg_t", [D, NG], F32R, isOutput=False)
    kt = nc.declare_dram_parameter("kt", [D, P], F32R, isOutput=False)
    wq_t = nc.declare_dram_parameter("wq_t", [D, D], F32R, isOutput=False)
    bq_col = nc.declare_dram_parameter("bq_col", [D, 1], F32, isOutput=False)
    vo = nc.declare_dram_parameter("vo", [NGRP, 128, D], BF16, isOutput=False)
    m01bd = nc.declare_dram_parameter("m01bd", [NGRP, 128, 128], BF16, isOutput=False)
    repm = nc.declare_dram_parameter("repm", [max(nrep, 1), 128, 128], BF16, isOutput=False)
    identb = nc.declare_dram_parameter("identb", [128, 128], BF16, isOutput=False)
    identf = nc.declare_dram_parameter("identf", [128, 128], F32, isOutput=False)
    identr = nc.declare_dram_parameter("identr", [128, 128], F32R, isOutput=False)
    emptyp = nc.declare_dram_parameter("emptyp", [128, 1], F32, isOutput=False)
    w1_t = nc.declare_dram_parameter("w1_t", [D, F1], BF16, isOutput=False)
    w2_t = nc.declare_dram_parameter("w2_t", [F1, D], BF16, isOutput=False)
    b1_col = nc.declare_dram_parameter("b1_col", [F1, 1], F32, isOutput=False)
    ln1_col = nc.declare_dram_parameter("ln1_col", [D, 2], F32, isOutput=False)
    gb_row = nc.declare_dram_parameter("gb_row", [4, D], F32, isOutput=False)
    out = nc.declare_dram_parameter("out", [B, NG, D], F32, isOutput=True)

    s_attn = 1.0 / float(np.sqrt(DH))
    any_empty = any(int(c) == 0 for c in counts)
    jobs = [b for b in range(B) if int(counts[b]) > 0]

    with TileContext(nc) as tc, nc.allow_low_precision(
            reason="bf16 matmul inputs; tolerance budget is 2e-2 of max"):
        import contextlib

        cstack = contextlib.ExitStack()
        consts = cstack.enter_context(tc.tile_pool(name="consts", bufs=1))

        # ---- persistent constants -------------------------------------
        def load_w(name, ap, rows, cols, dt):
            tiles = []
            for k in range(rows // 128):
                tl = consts.tile([128, cols], dt, tag=f"{name}{k}", name=f"{name}{k}")
                nc.sync.dma_start(out=tl[:], in_=ap[k * 128 : (k + 1) * 128, :])
                tiles.append(tl)
            return tiles

        hgb_sb = load_w("hgb", hgb_row, NG, D, F32R)  # row tiles (residual)
        hgt_sb = load_w("hgt", hg_t, D, NG, F32R)
        kt_sb = load_w("kt", kt, D, P, F32R)
        wq_sb = load_w("wq", wq_t, D, D, F32R)
        w1_sb = load_w("w1", w1_t, D, F1, BF16)
        w2_sb = load_w("w2", w2_t, F1, D, BF16)
        hg_sb = load_w("hg", hg_row, NG, D, F32R) if any_empty else None

        vo_sb = []
        for g in range(NGRP):
            tl = consts.tile([128, D], BF16, tag=f"vo{g}", name=f"vo{g}")
            nc.sync.dma_start(out=tl[:], in_=vo[g, :, :])
            vo_sb.append(tl)
        m01_sb = []
        for g in range(NGRP):
            tl = consts.tile([128, 128], BF16, tag=f"m01{g}", name=f"m01{g}")
            nc.sync.dma_start(out=tl[:], in_=m01bd[g, :, :])
            m01_sb.append(tl)
        rep_sb = []
        for i in range(nrep):
            tl = consts.tile([128, 128], BF16, tag=f"rep{i}", name=f"rep{i}")
            nc.sync.dma_start(out=tl[:], in_=repm[i, :, :])
            rep_sb.append(tl)
        identb_sb = consts.tile([128, 128], BF16, tag="identb", name="identb")
        nc.sync.dma_start(out=identb_sb[:], in_=identb[:, :])
        identf_sb = consts.tile([128, 128], F32, tag="identf", name="identf")
        nc.sync.dma_start(out=identf_sb[:], in_=identf[:, :])
        identr_sb = consts.tile([128, 128], F32R, tag="identr", name="identr")
        nc.sync.dma_start(out=identr_sb[:], in_=identr[:, :])
        empty_sb = consts.tile([128, 1], F32, tag="empty", name="empty")
        nc.sync.dma_start(out=empty_sb[:], in_=emptyp[:, :])
        b1_sb = load_w("b1c", b1_col, F1, 1, F32) if use_b1 else None
        bq_sb = load_w("bqc", bq_col, D, 1, F32) if use_bq else None
        ln1_sb = load_w("ln1c", ln1_col, D, 2, F32) if (use_g1 or use_b1ln) else None
        gbr_sb = None
        if use_g2 or use_b2ln or use_b2:
            gbr_sb = consts.tile([128, 4, D], F32, tag="gbr", name="gbr")
            nc.gpsimd.dma_start(out=gbr_sb[:], in_=gb_row[:, :].to_broadcast((128, 4, D)))

        # persistent activations
        qT_sb = [consts.tile([128, NG], F32R, tag=f"qT{i}", name=f"qT{i}")
                 for i in range(2)]
        Et = consts.tile([128, H, NG], BF16, tag="Et", name="Et")
        Eg = [consts.tile([128, NG], BF16, tag=f"Eg{g}", name=f"Eg{g}")
              for g in range(NGRP)]
        den_sb = consts.tile([128, NG], F32, tag="den", name="den")
        denr_sb = consts.tile([128, NG], BF16, tag="denr", name="denr")

        # ================= Phase A: shared projections ==================
        with tc.tile_pool(name="psA", bufs=2, space="PSUM") as psA:
            # qT [D, NG]
            for m in range(2):
                for c in range(NCH):
                    ps = psA.tile([128, CH], F32, tag="ps", name="ps")
                    for kk in range(2):
                        nc.tensor.matmul(
                            ps[:],
                            wq_sb[kk][:, m * 128 : (m + 1) * 128],
                            hgt_sb[kk][:, c * CH : (c + 1) * CH],
                            start=(kk == 0), stop=(kk == 1))
                    if use_bq:
                        nc.scalar.activation(
                            qT_sb[m][:, c * CH : (c + 1) * CH], ps[:], AF.Identity,
                            bias=bq_sb[m][:, 0:1])
                    else:
                        nc.scalar.activation(
                            qT_sb[m][:, c * CH : (c + 1) * CH], ps[:], AF.Copy)

            # E^T per head: Et[p, h, n] = exp(s * k_h q_h^T)
            for h in range(H):
                ktt = kt_sb[h // 4]
                for c in range(NCH):
                    ps = psA.tile([128, CH], F32, tag="ps", name="ps")
                    nc.tensor.matmul(
                        ps[:],
                        ktt[(h % 4) * DH : (h % 4 + 1) * DH, :],
                        qT_sb[h // 4][(h % 4) * DH : (h % 4 + 1) * DH,
                                      c * CH : (c + 1) * CH],
                        start=True, stop=True,
                        tile_position=((h % 4) * DH, 0))
                    nc.scalar.activation(Et[:, h, c * CH : (c + 1) * CH],
                                         ps[:], AF.Exp, scale=s_attn)

            # regroup E into per-block layout [(h,p16), n]
            for g in range(NGRP):
                for h in range(H):
                    nc.sync.dma_start(
                        out=Eg[g][h * GW : (h + 1) * GW, :],
                        in_=Et[g * GW : (g + 1) * GW, h, :])

            # softmax denominators, packed [(h,b), n]; +1 on empty batches
            for c in range(NCH):
                psd = psA.tile([128, CH], F32, tag="psd", name="psd")
                for g in range(NGRP):
                    nc.tensor.matmul(
                        psd[:], m01_sb[g][:], Eg[g][:, c * CH : (c + 1) * CH],
                        start=(g == 0), stop=(g == NGRP - 1))
                nc.vector.tensor_scalar(
                    out=den_sb[:, c * CH : (c + 1) * CH], in0=psd[:],
                    scalar1=empty_sb[:, 0:1], scalar2=None, op0=ALU.add)
            nc.vector.reciprocal(out=denr_sb[:], in_=den_sb[:])

        # ================= Phase B: per-batch back half =================
        # Emission is interleaved at 3-tile-group granularity so the PE
        # always has FFN work of batch j-1 queued behind the attention/LN1
        # work of batch j (in-order engine queues; stalls otherwise).
        work = cstack.enter_context(tc.tile_pool(name="work", bufs=2))
        ehpool = cstack.enter_context(tc.tile_pool(name="ehp", bufs=2))
        ps_ao = cstack.enter_context(tc.tile_pool(name="ps_ao", bufs=3, space="PSUM"))
        ps_tp = cstack.enter_context(tc.tile_pool(name="ps_tp", bufs=3, space="PSUM"))
        ps_ff = cstack.enter_context(tc.tile_pool(name="ps_ff", bufs=2, space="PSUM"))

        RC2, RC1, RC0 = 0.29333931447269, -1.1711876763158582, 1.8939170369253155

        def rsqrt_group(vv_ap, mu_ap, sc_ap, ng_ap, sA, sB):
            """sc = rsqrt(vv), ng = -mu*sc on [128, 3] slices. Quadratic seed
            + one Newton step; tensor_tensor ops run on the idle Pool engine,
            imm tensor_scalar ops on DVE (Pool has no TensorScalar)."""
            nc.vector.tensor_scalar(out=sA, in0=vv_ap, scalar1=RC2,
                                    scalar2=RC1, op0=ALU.mult, op1=ALU.add)
            nc.gpsimd.tensor_tensor(out=sA, in0=sA, in1=vv_ap, op=ALU.mult)
            nc.vector.tensor_scalar(out=sA, in0=sA, scalar1=RC0,
                                    scalar2=None, op0=ALU.add)
            nc.gpsimd.tensor_tensor(out=sB, in0=sA, in1=sA, op=ALU.mult)
            nc.gpsimd.tensor_tensor(out=sB, in0=sB, in1=vv_ap, op=ALU.mult)
            nc.vector.tensor_scalar(out=sB, in0=sB, scalar1=-0.5,
                                    scalar2=1.5, op0=ALU.mult, op1=ALU.add)
            nc.gpsimd.tensor_tensor(out=sc_ap, in0=sA, in1=sB, op=ALU.mult)
            nc.vector.scalar_tensor_tensor(
                out=ng_ap, in0=mu_ap, scalar=-1.0, in1=sc_ap,
                op0=ALU.mult, op1=ALU.mult)

        def front_attn(b, bl):
            """Replicated+masked reciprocal denominators -> Ehat per block."""
            eh = []
            for i, (g, ri) in enumerate(bl):
                tl = ehpool.tile([128, NG], BF16, tag=f"eh{i}", name=f"eh{i}")
                eh.append(tl)
            for c in range(NCH):
                for i, (g, ri) in enumerate(bl):
                    psr = ps_tp.tile([128, CH], F32, tag="tp", name="rep")
                    nc.tensor.matmul(
                        psr[:], rep_sb[ri][:],
                        denr_sb[:, c * CH : (c + 1) * CH],
                        start=True, stop=True)
                    nc.vector.tensor_mul(
                        eh[i][:, c * CH : (c + 1) * CH],
                        Eg[g][:, c * CH : (c + 1) * CH], psr[:])
            return eh

        def front_new(b):
            st = {
                "mvb": work.tile([128, NT, 2], F32, tag="mvb", name="mvb"),
                "sc1": work.tile([128, NT], F32, tag="sc1", name="sc1"),
                "ng1": work.tile([128, NT], F32, tag="ng1", name="ng1"),
                "stats": work.tile([128, 6], F32, tag="stats", name="stats"),
                "x_row": work.tile([128, NT, D], BF16, tag="x_row", name="x_row"),
                "xT": work.tile([128, 2, NG], BF16, tag="xT", name="xT"),
                "sA": work.tile([128, 3], F32, tag="sA", name="sA"),
                "sB": work.tile([128, 3], F32, tag="sB", name="sB"),
                "pg": {},
            }
            return st

        def front_group(b, bl, st, c):
            """Attention matmuls + H residual + LN1 stats for tiles 3c..3c+2."""
            hrow = hgb_sb if (b is not None) else (hg_sb or hgb_sb)
            for t in range(c * 3, c * 3 + 3):
                psa = ps_ao.tile([128, D], F32, tag="ao", name="ao")
                if bl:
                    for i, (g, ri) in enumerate(bl):
                        nc.tensor.matmul(
                            psa[:], st["eh"][i][:, t * 128 : (t + 1) * 128],
                            vo_sb[g][:], start=(i == 0), stop=False)
                nc.tensor.matmul(psa[:], identr_sb[:], hrow[t][:],
                                 start=(not bl), stop=True)
                nc.vector.bn_stats(out=st["stats"][:], in_=psa[:])
                nc.vector.bn_aggr(out=st["mvb"][:, t, :], in_=st["stats"][:])
                st["pg"][t] = psa

        def front_apply(b, st, c):
            """rsqrt + LN1 apply + transpose to xT for tiles 3c..3c+2."""
            t0 = c * 3
            mvb, sc1, ng1 = st["mvb"], st["sc1"], st["ng1"]
            rsqrt_group(mvb[:, t0 : t0 + 3, 1], mvb[:, t0 : t0 + 3, 0],
                        sc1[:, t0 : t0 + 3], ng1[:, t0 : t0 + 3],
                        st["sA"][:], st["sB"][:])
            for t in range(t0, t0 + 3):
                nc.scalar.activation(
                    st["x_row"][:, t, :], st["pg"].pop(t)[:], AF.Identity,
                    bias=ng1[:, t : t + 1], scale=sc1[:, t : t + 1])
                psx = ps_tp.tile([128, D], BF16, tag="tp", name="xp")
                for k in range(2):
                    nc.tensor.transpose(
                        psx[:, k * 128 : (k + 1) * 128],
                        st["x_row"][:, t, k * 128 : (k + 1) * 128],
                        identb_sb[:])
                nc.vector.tensor_copy(
                    out=st["xT"][:, :, t * 128 : (t + 1) * 128],
                    in_=psx[:].rearrange("p (k n) -> p k n", k=2))

        def back_ffn1(b, st, c):
            if c == 0:
                st["h1g"] = work.tile([128, 8, NG], BF16, tag="h1g", name="h1g")
            xT = st["xT"]
            for m in range(8):
                ps = ps_ff.tile([128, CH], F32, tag="ff", name="f1")
                for kk in range(2):
                    nc.tensor.matmul(
                        ps[:], w1_sb[kk][:, m * 128 : (m + 1) * 128],
                        xT[:, kk, c * CH : (c + 1) * CH],
                        start=(kk == 0), stop=(kk == 1))
                if use_b1:
                    nc.scalar.activation(st["h1g"][:, m, c * CH : (c + 1) * CH],
                                         ps[:], AF.Gelu, bias=b1_sb[m][:, 0:1])
                else:
                    nc.scalar.activation(st["h1g"][:, m, c * CH : (c + 1) * CH],
                                         ps[:], AF.Gelu)

        def back_ffn2_ln2(b, st):
            # FFN2 (T layout) + x residual accumulated on the PE; ACT evac
            xres = st["xT"]
            if use_g1 or use_b1ln:
                xres = work.tile([128, 2, NG], BF16, tag="xres", name="xres")
                for k in range(2):
                    nc.vector.tensor_scalar(
                        out=xres[:, k, :], in0=st["xT"][:, k, :],
                        scalar1=ln1_sb[k][:, 0:1], scalar2=ln1_sb[k][:, 1:2],
                        op0=ALU.mult, op1=ALU.add)
            yT = work.tile([128, 2, NG], BF16, tag="yT", name="yT")
            for c in range(NCH):
                for m in range(2):
                    ps = ps_ff.tile([128, CH], F32, tag="ff", name="y")
                    for kk in range(8):
                        nc.tensor.matmul(
                            ps[:], w2_sb[kk][:, m * 128 : (m + 1) * 128],
                            st["h1g"][:, kk, c * CH : (c + 1) * CH],
                            start=(kk == 0), stop=False)
                    nc.tensor.matmul(
                        ps[:], identb_sb[:], xres[:, m, c * CH : (c + 1) * CH],
                        start=False, stop=True)
                    nc.scalar.activation(yT[:, m, c * CH : (c + 1) * CH],
                                         ps[:], AF.Copy)

            mv2 = work.tile([128, NT, 2], F32, tag="mv2", name="mv2")
            sc2 = work.tile([128, NT], F32, tag="sc2", name="sc2")
            ng2 = work.tile([128, NT], F32, tag="ng2", name="ng2")
            st2 = work.tile([128, 6], F32, tag="st2", name="st2")
            sA2 = work.tile([128, 3], F32, tag="sA2", name="sA2")
            sB2 = work.tile([128, 3], F32, tag="sB2", name="sB2")
            for c in range(NCH):
                t0 = c * 3
                pg = []
                for t in range(t0, t0 + 3):
                    psy = ps_tp.tile([128, D], BF16, tag="tp", name="yt")
                    for k in range(2):
                        nc.tensor.transpose(
                            psy[:, k * 128 : (k + 1) * 128],
                            yT[:, k, t * 128 : (t + 1) * 128], identb_sb[:])
                    if use_b2:
                        nc.vector.tensor_add(psy[:], psy[:], gbr_sb[:, 0, :])
                    nc.vector.bn_stats(out=st2[:], in_=psy[:])
                    nc.vector.bn_aggr(out=mv2[:, t, :], in_=st2[:])
                    pg.append(psy)
                rsqrt_group(mv2[:, t0 : t0 + 3, 1], mv2[:, t0 : t0 + 3, 0],
                            sc2[:, t0 : t0 + 3], ng2[:, t0 : t0 + 3],
                            sA2[:], sB2[:])
                for t in range(t0, t0 + 3):
                    orow = work.tile([128, D], F32, tag=f"or{t % 3}", name="orow")
                    nc.scalar.activation(
                        orow[:], pg[t - t0][:], AF.Identity,
                        bias=ng2[:, t : t + 1], scale=sc2[:, t : t + 1])
                    if use_g2:
                        nc.vector.tensor_mul(orow[:], orow[:], gbr_sb[:, 2, :])
                    if use_b2ln:
                        nc.vector.tensor_add(orow[:], orow[:], gbr_sb[:, 3, :])
                    if b is not None:
                        nc.sync.dma_start(
                            out=out[b, t * 128 : (t + 1) * 128, :], in_=orow[:])
                    else:
                        for be in range(B):
                            if int(counts[be]) == 0:
                                nc.sync.dma_start(
                                    out=out[be, t * 128 : (t + 1) * 128, :],
                                    in_=orow[:])

        alljobs = [(b, blocks[b]) for b in jobs]
        if any_empty:
            alljobs.append((None, []))
        prev = None
        for b, bl in alljobs:
            st = front_new(b)
            st["eh"] = front_attn(b, bl) if bl else []
            front_group(b, bl, st, 0)
            if prev is not None:
                back_ffn1(prev[0], prev[1], 0)
            front_apply(b, st, 0)
            front_group(b, bl, st, 1)
            if prev is not None:
                back_ffn1(prev[0], prev[1], 1)
            front_apply(b, st, 1)
            if prev is not None:
                back_ffn2_ln2(prev[0], prev[1])
            prev = (b, st)
        back_ffn1(prev[0], prev[1], 0)
        back_ffn1(prev[0], prev[1], 1)
        back_ffn2_ln2(prev[0], prev[1])

        cstack.close()

    return nc


def kernel(H_genes, perturbation_indices, batch_assignment, batch_size,
           in_proj_w, in_proj_b, out_proj_w, out_proj_b,
           ffn_w1, ffn_b1, ffn_w2, ffn_b2,
           ln1_g, ln1_b, ln2_g, ln2_b):
    import ml_dtypes
    bf16 = ml_dtypes.bfloat16

    Hg = np.ascontiguousarray(np.asarray(H_genes, dtype=np.float32))
    pidx = np.asarray(perturbation_indices).astype(np.int64)
    ba = np.asarray(batch_assignment).astype(np.int64)
    Bs = int(np.asarray(batch_size))
    assert Bs == B, f"kernel hardcodes B=16, got {Bs}"
    assert Hg.shape == (N, D)

    Wq, Wk, Wv = [np.asarray(w, np.float32) for w in np.split(np.asarray(in_proj_w), 3, axis=0)]
    bq, bk, bv = [np.asarray(x, np.float32) for x in np.split(np.asarray(in_proj_b), 3, axis=0)]
    Wo = np.asarray(out_proj_w, np.float32)
    bo = np.asarray(out_proj_b, np.float32)
    W1 = np.asarray(ffn_w1, np.float32)
    b1 = np.asarray(ffn_b1, np.float32)
    W2 = np.asarray(ffn_w2, np.float32)
    b2 = np.asarray(ffn_b2, np.float32)
    g1 = np.asarray(ln1_g, np.float32)
    be1 = np.asarray(ln1_b, np.float32)
    g2 = np.asarray(ln2_g, np.float32)
    be2 = np.asarray(ln2_b, np.float32)

    counts = np.bincount(ba, minlength=B).astype(np.int64)
    has_any = counts > 0

    # host-side small projections: k and Wo-folded values
    Hp = Hg[pidx]                                   # [P, D]
    k = Hp @ Wk.T + bk[None, :]                     # [P, D]
    v = Hp @ Wv.T + bv[None, :]                     # [P, D]
    # vo[(h,p),:] = v[p, h-slice] @ Wo[:, h-slice].T  (full attn_out proj)
    vo = np.zeros((NGRP, 128, D), np.float32)
    for g in range(NGRP):
        for h in range(H):
            vh = v[g * GW : (g + 1) * GW, h * DH : (h + 1) * DH]   # [16, 32]
            vo[g, h * GW : (h + 1) * GW, :] = vh @ Wo[:, h * DH : (h + 1) * DH].T

    # block-diagonal per-head mask matrices for denominators
    m01 = (ba[:, None] == np.arange(B)[None, :]).astype(np.float32)  # [P, B]
    m01bd = np.zeros((NGRP, 128, 128), np.float32)
    for g in range(NGRP):
        for h in range(H):
            m01bd[g, h * GW : (h + 1) * GW, h * GW : (h + 1) * GW] = \
                m01[g * GW : (g + 1) * GW, :]

    # replication matrices: rep[(h,b'),(h,p16)] = 1{b'==b and ba[p]==b}
    blocks = {b: [] for b in range(B)}
    rep_mats = []
    for b in range(B):
        if counts[b] == 0:
            continue
        for g in range(NGRP):
            psel = np.where(ba[g * GW : (g + 1) * GW] == b)[0]
            if len(psel) == 0:
                continue
            R = np.zeros((128, 128), np.float32)
            for h in range(H):
                for p16 in psel:
                    R[h * GW + b, h * GW + p16] = 1.0
            blocks[b].append((g, len(rep_mats)))
            rep_mats.append(R)
    nrep = len(rep_mats)

    # fold ln1 gain into FFN1 (exact): W1' = W1*g1, b1' = W1@b1_ln + b1
    W1f = W1 * g1[None, :]
    b1f = b1 + W1 @ be1

    Hg_pad = np.zeros((NPAD, D), np.float32)
    Hg_pad[:N] = Hg
    emptyp = np.tile((~has_any).astype(np.float32), H)[:, None]  # [(h,b),1]

    flags = (
        bool(np.any(bq != 0)), bool(np.any(b1f != 0)), bool(np.any(b2 != 0)),
        bool(np.any(g1 != 1)), bool(np.any(be1 != 0)),
        bool(np.any(g2 != 1)), bool(np.any(be2 != 0)),
    )
    use_bo = bool(np.any(bo != 0))

    nc = _build_program(counts, blocks, nrep, flags)

    common = {
        "kt": np.ascontiguousarray(k.T),
        "wq_t": np.ascontiguousarray(Wq.T),
        "bq_col": bq[:, None].copy(),
        "vo": vo.astype(bf16),
        "m01bd": m01bd.astype(bf16),
        "repm": (np.stack(rep_mats) if nrep else np.zeros((1, 128, 128), np.float32)).astype(bf16),
        "identb": np.eye(128, dtype=np.float32).astype(bf16),
        "identf": np.eye(128, dtype=np.float32),
        "identr": np.eye(128, dtype=np.float32),
        "emptyp": np.ascontiguousarray(emptyp),
        "w1_t": np.ascontiguousarray(W1f.T).astype(bf16),
        "w2_t": np.ascontiguousarray(W2.T).astype(bf16),
        "b1_col": b1f[:, None].copy(),
        "ln1_col": np.ascontiguousarray(np.stack([g1, be1], axis=1)),
        "gb_row": np.stack([b2, be1, g2, be2], axis=0),
    }
    in_maps = []
    for c in range(NCORES):
        sl = Hg_pad[c * NG : (c + 1) * NG]
        m = dict(common)
        m["hg_row"] = np.ascontiguousarray(sl)
        m["hgb_row"] = np.ascontiguousarray(sl + bo[None, :]) if use_bo else m["hg_row"]
        m["hg_t"] = np.ascontiguousarray(sl.T)
        in_maps.append(m)

    if os.environ.get("BASS_KERNEL_SIM"):
        from concourse import bass_interp
        # CoreSim lacks a Gelu LUT; shim exact (erf) gelu for local debugging.
        if not getattr(bass_interp.InstructionExecutor, "_gelu_patched", False):
            from scipy.special import erf
            _orig_act = bass_interp.InstructionExecutor.visit_InstActivation

            def _act(self, instruction, *, reg_snapshot=None):
                if instruction.func == mybir.ActivationFunctionType.Gelu:
                    instruction.func = mybir.ActivationFunctionType.Identity
                    try:
                        import concourse.bass_interp as bi
                        out_ap = instruction.outs[0]
                        r = _orig_act(self, instruction, reg_snapshot=reg_snapshot)
                        view = self.view_ap(out_ap, bi.Direction.READ, instruction,
                                            reg_snapshot=reg_snapshot)
                        x = view.astype(np.float64)
                        view[:] = (0.5 * x * (1.0 + erf(x / np.sqrt(2.0)))).astype(view.dtype)
                        return r
                    finally:
                        instruction.func = mybir.ActivationFunctionType.Gelu
                return _orig_act(self, instruction, reg_snapshot=reg_snapshot)

            bass_interp.InstructionExecutor.visit_InstActivation = _act
            bass_interp.InstructionExecutor._gelu_patched = True
        nsim = int(os.environ.get("BASS_KERNEL_SIM_CORES", "1"))
        simtrace = bool(os.environ.get("BASS_KERNEL_SIMTRACE"))
        sim = bass_interp.MultiCoreSim(nc, nsim, trace=simtrace)
        for c in range(nsim):
            for kk, vv in in_maps[c].items():
                sim.cores[c].tensor(kk)[:] = vv
        sim.simulate()
        print(f"SIM predicted time: {sim.cores[0].time} ns")
        full = np.zeros((B, NPAD, D), np.float32)
        for c in range(nsim):
            full[:, c * NG : (c + 1) * NG, :] = (
                np.array(sim.cores[c].mem_tensor("out")).reshape(B, NG, D))
        return full[:, :N, :]

    from concourse.bass_utils import run_bass_kernel_spmd
    _split_waits(nc)
    trace = bool(os.environ.get("BASS_KERNEL_TRACE"))
    res = run_bass_kernel_spmd(nc, in_maps, core_ids=list(range(NCORES)),
                               trace=trace)
    if trace and res.exec_time_ns is not None:
        print(f"HW exec time: {res.exec_time_ns} ns")
        if res.instructions_and_trace:
            print("trace:", res.instructions_and_trace[1])

    full = np.zeros((B, NPAD, D), np.float32)
    for c in range(NCORES):
        full[:, c * NG : (c + 1) * NG, :] = res.results[c]["out"]
    return full[:, :N, :]


# revision 25
# speedup vs baseline: 1.9489x; 1.0060x over previous
"""Trainium2 Bass kernel for nn_EquivariantPerturbationTransform.

Reference (N=6000 genes, D=256, H=8 heads, P=128 perturbations, B=16):
  q = H @ Wq.T ; k,v from gathered perturbation rows
  scores[h,n,p] shared across batches; per-batch mask over p (ragged)
  attn_out[b] = softmax-masked attention -> out proj (zeroed for empty b)
  x = LN1(H + attn_out); out = LN2(x + gelu(x@W1.T)@W2.T)

v2 strategy (sequence-parallel over 8 cores, 768 query rows/core):
  - Wo is folded into the values on the host: vo[(h,p),:] = v[p,h] @ Wo_h.T,
    so attention context IS the projected attn_out -- the per-batch [N,D]@[D,D]
    out-projection disappears and the attention matmul lands in row layout.
  - Normalized masked attention weights built per (batch, 16-block):
    a replication matmul (0/1 stationary R_bg) broadcasts reciprocal softmax
    denominators from [(h,b),n] packed layout into [(h,p16),n] block layout
    (zeroing masked rows), then one gpsimd multiply gives Ehat = E * rep.
  - LN1 in row layout straight from the attention PSUM; x transposed (bf16,
    1 cyc/row) to T layout for FFN1; FFN2 in T layout; the PSUM->SBUF evac of
    y adds the x residual for free (gpsimd tensor_add); y transposed back for
    LN2 + store.
  - rsqrt for the LNs via a single DVE tensor_scalar (add eps, pow -0.5) --
    keeps Sqrt off the ACT engine so the activation table never leaves
    gelu_and_others (Gelu/Identity/Copy) after the phase-A Exp.
  - bf16 matmuls everywhere except q/k scores (fp32r: exp amplifies score
    rounding); fp32 PSUM accumulation; fp32 H residual into LN1.
  - Engine split: PE matmuls/transposes; DVE LN stats+applies+recip; gpsimd
    Ehat muls, r1 adds, xT copies, y evac+residual; ACT exp+gelu only.
"""

import os
import sys

sys.path.insert(0, "/opt/trn_rl_repo")

import numpy as np

import concourse.bass as bass
from concourse import mybir
from concourse.tile import TileContext

F32 = mybir.dt.float32
F32R = mybir.dt.float32r
BF16 = mybir.dt.bfloat16
AF = mybir.ActivationFunctionType
ALU = mybir.AluOpType

N, D, H, P, B = 6000, 256, 8, 128, 16
DH = D // H  # 32
NCORES = 8
NPAD = 6144
NG = NPAD // NCORES  # 768 rows per core
NT = NG // 128       # 6 row tiles
NCH = 2              # 384-wide chunks
CH = NG // NCH
EPS = 1e-5
GW = 16              # perturbation block width
NGRP = P // GW       # 8 blocks
F1 = 4 * D           # 1024


def _split_waits(nc, max_waits=1):
    """The neuronxcc/walrus build here rejects >1 sync-wait per instruction;
    hoist excess waits onto same-engine NoOps (semantically identical)."""
    n_split = 0
    for f in nc.m.functions:
        for bb in f.blocks:
            new_list = []
            for ins in bb.instructions:
                si = getattr(ins, "sync_info", None)
                if si is not None and si.on_wait and len(si.on_wait) > max_waits:
                    waits = list(si.on_wait)
                    excess, keep = waits[:-max_waits], waits[-max_waits:]
                    for i in range(0, len(excess), max_waits):
                        chunk = excess[i : i + max_waits]
                        nop = mybir.InstNoOp(name=f"{ins.name}-ws{i}", ins=[], outs=[])
                        nop.engine = ins.engine
                        nop.sync_info = mybir.SyncInfo(on_wait=chunk, on_update=[])
                        new_list.append(nop)
                        n_split += 1
                    si.on_wait = keep
                new_list.append(ins)
            bb.instructions = new_list
    return n_split


def _build_program(counts, blocks, nrep, flags):
    """blocks[b] = list of (g, rep_idx) block/replication-matrix pairs."""
    (use_bq, use_b1, use_b2, use_g1, use_b1ln, use_g2, use_b2ln) = flags
    nc = bass.Bass()

    # ---- DRAM parameters -------------------------------------------------
    hgb_row = nc.declare_dram_parameter("hgb_row", [NG, D], F32R, isOutput=False)
    hg_row = nc.declare_dram_parameter("hg_row", [NG, D], F32R, isOutput=False)
    hg_t = nc.declare_dram_parameter("hg_t", [D, NG], F32R, isOutput=False)
    kt = nc.declare_dram_parameter("kt", [D, P], F32R, isOutput=False)
    wq_t = nc.declare_dram_parameter("wq_t", [D, D], F32R, isOutput=False)
    bq_col = nc.declare_dram_parameter("bq_col", [D, 1], F32, isOutput=False)
    vo = nc.declare_dram_parameter("vo", [NGRP, 128, D], BF16, isOutput=False)
    m01bd = nc.declare_dram_parameter("m01bd", [NGRP, 128, 128], BF16, isOutput=False)
    repm = nc.declare_dram_parameter("repm", [max(nrep, 1), 128, 128], BF16, isOutput=False)
    identb = nc.declare_dram_parameter("identb", [128, 128], BF16, isOutput=False)
    identf = nc.declare_dram_parameter("identf", [128, 128], F32, isOutput=False)
    identr = nc.declare_dram_parameter("identr", [128, 128], F32R, isOutput=False)
    emptyp = nc.declare_dram_parameter("emptyp", [128, 1], F32, isOutput=False)
    w1_t = nc.declare_dram_parameter("w1_t", [D, F1], BF16, isOutput=False)
    w2_t = nc.declare_dram_parameter("w2_t", [F1, D], BF16, isOutput=False)
    b1_col = nc.declare_dram_parameter("b1_col", [F1, 1], F32, isOutput=False)
    ln1_col = nc.declare_dram_parameter("ln1_col", [D, 2], F32, isOutput=False)
    gb_row = nc.declare_dram_parameter("gb_row", [4, D], F32, isOutput=False)
    out = nc.declare_dram_parameter("out", [B, NG, D], F32, isOutput=True)

    s_attn = 1.0 / float(np.sqrt(DH))
    any_empty = any(int(c) == 0 for c in counts)
    jobs = [b for b in range(B) if int(counts[b]) > 0]

    with TileContext(nc) as tc, nc.allow_low_precision(
            reason="bf16 matmul inputs; tolerance budget is 2e-2 of max"):
        import contextlib
        import itertools

        _dmaq = itertools.cycle(
            lambda o, i: nc.sync.dma_start(out=o, in_=i)
            for _ in range(1))
        _dma_engines = None

        def dma(out_ap, in_ap):
            nonlocal _dma_engines
            if _dma_engines is None:
                _dma_engines = itertools.cycle(
                    [nc.sync, nc.gpsimd, nc.scalar])
            next(_dma_engines).dma_start(out=out_ap, in_=in_ap)

        cstack = contextlib.ExitStack()
        consts = cstack.enter_context(tc.tile_pool(name="consts", bufs=1))

        # ---- persistent constants -------------------------------------
        def load_w(name, ap, rows, cols, dt):
            tiles = []
            for k in range(rows // 128):
                tl = consts.tile([128, cols], dt, tag=f"{name}{k}", name=f"{name}{k}")
                dma(tl[:], ap[k * 128 : (k + 1) * 128, :])
                tiles.append(tl)
            return tiles

        hgb_sb = load_w("hgb", hgb_row, NG, D, F32R)  # row tiles (residual)
        hgt_sb = load_w("hgt", hg_t, D, NG, F32R)
        kt_sb = load_w("kt", kt, D, P, F32R)
        wq_sb = load_w("wq", wq_t, D, D, F32R)
        w1_sb = load_w("w1", w1_t, D, F1, BF16)
        w2_sb = load_w("w2", w2_t, F1, D, BF16)
        hg_sb = load_w("hg", hg_row, NG, D, F32R) if any_empty else None

        vo_sb = []
        for g in range(NGRP):
            tl = consts.tile([128, D], BF16, tag=f"vo{g}", name=f"vo{g}")
            dma(tl[:], vo[g, :, :])
            vo_sb.append(tl)
        m01_sb = []
        for g in range(NGRP):
            tl = consts.tile([128, 128], BF16, tag=f"m01{g}", name=f"m01{g}")
            dma(tl[:], m01bd[g, :, :])
            m01_sb.append(tl)
        rep_sb = []
        for i in range(nrep):
            tl = consts.tile([128, 128], BF16, tag=f"rep{i}", name=f"rep{i}")
            dma(tl[:], repm[i, :, :])
            rep_sb.append(tl)
        identb_sb = consts.tile([128, 128], BF16, tag="identb", name="identb")
        dma(identb_sb[:], identb[:, :])
        identf_sb = consts.tile([128, 128], F32, tag="identf", name="identf")
        dma(identf_sb[:], identf[:, :])
        identr_sb = consts.tile([128, 128], F32R, tag="identr", name="identr")
        dma(identr_sb[:], identr[:, :])
        empty_sb = consts.tile([128, 1], F32, tag="empty", name="empty")
        dma(empty_sb[:], emptyp[:, :])
        b1_sb = load_w("b1c", b1_col, F1, 1, F32) if use_b1 else None
        bq_sb = load_w("bqc", bq_col, D, 1, F32) if use_bq else None
        ln1_sb = load_w("ln1c", ln1_col, D, 2, F32) if (use_g1 or use_b1ln) else None
        gbr_sb = None
        if use_g2 or use_b2ln or use_b2:
            gbr_sb = consts.tile([128, 4, D], F32, tag="gbr", name="gbr")
            nc.gpsimd.dma_start(out=gbr_sb[:], in_=gb_row[:, :].to_broadcast((128, 4, D)))

        # persistent activations
        qT_sb = [consts.tile([128, NG], F32R, tag=f"qT{i}", name=f"qT{i}")
                 for i in range(2)]
        Et = consts.tile([128, H, NG], BF16, tag="Et", name="Et")
        Eg = [consts.tile([128, NG], BF16, tag=f"Eg{g}", name=f"Eg{g}")
              for g in range(NGRP)]
        den_sb = consts.tile([128, NG], F32, tag="den", name="den")
        denf_sb = consts.tile([128, NG], F32, tag="denf", name="denf")
        denr_sb = consts.tile([128, NG], BF16, tag="denr", name="denr")

        # ================= Phase A: shared projections ==================
        with tc.tile_pool(name="psA", bufs=2, space="PSUM") as psA:
            # qT [D, NG]
            for m in range(2):
                for c in range(NCH):
                    ps = psA.tile([128, CH], F32, tag="ps", name="ps")
                    for kk in range(2):
                        nc.tensor.matmul(
                            ps[:],
                            wq_sb[kk][:, m * 128 : (m + 1) * 128],
                            hgt_sb[kk][:, c * CH : (c + 1) * CH],
                            start=(kk == 0), stop=(kk == 1))
                    if use_bq:
                        nc.scalar.activation(
                            qT_sb[m][:, c * CH : (c + 1) * CH], ps[:], AF.Identity,
                            bias=bq_sb[m][:, 0:1])
                    else:
                        nc.scalar.activation(
                            qT_sb[m][:, c * CH : (c + 1) * CH], ps[:], AF.Copy)

            # E^T per head: Et[p, h, n] = exp(s * k_h q_h^T); each head's
            # block-layout regroup DMAs are issued as soon as it finishes.
            for h in range(H):
                ktt = kt_sb[h // 4]
                for c in range(NCH):
                    ps = psA.tile([128, CH], F32, tag="ps", name="ps")
                    nc.tensor.matmul(
                        ps[:],
                        ktt[(h % 4) * DH : (h % 4 + 1) * DH, :],
                        qT_sb[h // 4][(h % 4) * DH : (h % 4 + 1) * DH,
                                      c * CH : (c + 1) * CH],
                        start=True, stop=True,
                        tile_position=((h % 4) * DH, 0))
                    nc.scalar.activation(Et[:, h, c * CH : (c + 1) * CH],
                                         ps[:], AF.Exp, scale=s_attn)
                for g in range(NGRP):
                    dma(Eg[g][h * GW : (h + 1) * GW, :],
                        Et[g * GW : (g + 1) * GW, h, :])

            # softmax denominators, packed [(h,b), n]; +1 on empty batches
            for c in range(NCH):
                psd = psA.tile([128, CH], F32, tag="psd", name="psd")
                for g in range(NGRP):
                    nc.tensor.matmul(
                        psd[:], m01_sb[g][:], Eg[g][:, c * CH : (c + 1) * CH],
                        start=(g == 0), stop=(g == NGRP - 1))
                nc.vector.tensor_scalar(
                    out=den_sb[:, c * CH : (c + 1) * CH], in0=psd[:],
                    scalar1=empty_sb[:, 0:1], scalar2=None, op0=ALU.add)
            nc.vector.reciprocal(out=denr_sb[:], in_=den_sb[:])

        # ================= Phase B: per-batch back half =================
        # Emission is interleaved at 3-tile-group granularity so the PE
        # always has FFN work of batch j-1 queued behind the attention/LN1
        # work of batch j (in-order engine queues; stalls otherwise).
        work = cstack.enter_context(tc.tile_pool(name="work", bufs=2))
        ehpool = cstack.enter_context(tc.tile_pool(name="ehp", bufs=2))
        ps_ao = cstack.enter_context(tc.tile_pool(name="ps_ao", bufs=3, space="PSUM"))
        ps_tp = cstack.enter_context(tc.tile_pool(name="ps_tp", bufs=3, space="PSUM"))
        ps_ff = cstack.enter_context(tc.tile_pool(name="ps_ff", bufs=2, space="PSUM"))

        RC2, RC1, RC0 = 0.29333931447269, -1.1711876763158582, 1.8939170369253155

        def rsqrt_group(vv_ap, mu_ap, sc_ap, ng_ap, sA, sB):
            """sc = rsqrt(vv), ng = -mu*sc on [128, 3] slices. Quadratic seed
            + one Newton step; tensor_tensor ops run on the idle Pool engine,
            imm tensor_scalar ops on DVE (Pool has no TensorScalar)."""
            nc.vector.tensor_scalar(out=sA, in0=vv_ap, scalar1=RC2,
                                    scalar2=RC1, op0=ALU.mult, op1=ALU.add)
            nc.gpsimd.tensor_tensor(out=sA, in0=sA, in1=vv_ap, op=ALU.mult)
            nc.vector.tensor_scalar(out=sA, in0=sA, scalar1=RC0,
                                    scalar2=None, op0=ALU.add)
            nc.gpsimd.tensor_tensor(out=sB, in0=sA, in1=sA, op=ALU.mult)
            nc.gpsimd.tensor_tensor(out=sB, in0=sB, in1=vv_ap, op=ALU.mult)
            nc.vector.tensor_scalar(out=sB, in0=sB, scalar1=-0.5,
                                    scalar2=1.5, op0=ALU.mult, op1=ALU.add)
            nc.gpsimd.tensor_tensor(out=sc_ap, in0=sA, in1=sB, op=ALU.mult)
            nc.vector.scalar_tensor_tensor(
                out=ng_ap, in0=mu_ap, scalar=-1.0, in1=sc_ap,
                op0=ALU.mult, op1=ALU.mult)

        def front_attn(b, bl):
            """Replicated+masked reciprocal denominators -> Ehat per block."""
            eh = []
            for i, (g, ri) in enumerate(bl):
                tl = ehpool.tile([128, NG], BF16, tag=f"eh{i}", name=f"eh{i}")
                eh.append(tl)
            for c in range(NCH):
                for i, (g, ri) in enumerate(bl):
                    psr = ps_tp.tile([128, CH], F32, tag="tp", name="rep")
                    nc.tensor.matmul(
                        psr[:], rep_sb[ri][:],
                        denr_sb[:, c * CH : (c + 1) * CH],
                        start=True, stop=True)
                    nc.vector.tensor_mul(
                        eh[i][:, c * CH : (c + 1) * CH],
                        Eg[g][:, c * CH : (c + 1) * CH], psr[:])
            return eh

        def front_new(b):
            st = {
                "mvb": work.tile([128, NT, 2], F32, tag="mvb", name="mvb"),
                "sc1": work.tile([128, NT], F32, tag="sc1", name="sc1"),
                "ng1": work.tile([128, NT], F32, tag="ng1", name="ng1"),
                "stats": work.tile([128, 6], F32, tag="stats", name="stats"),
                "x_row": work.tile([128, NT, D], BF16, tag="x_row", name="x_row"),
                "xT": work.tile([128, 2, NG], BF16, tag="xT", name="xT"),
                "sA": work.tile([128, 3], F32, tag="sA", name="sA"),
                "sB": work.tile([128, 3], F32, tag="sB", name="sB"),
                "pg": {},
            }
            return st

        def front_group(b, bl, st, c):
            """Attention matmuls + H residual + LN1 stats for tiles 3c..3c+2."""
            hrow = hgb_sb if (b is not None) else (hg_sb or hgb_sb)
            for t in range(c * 3, c * 3 + 3):
                psa = ps_ao.tile([128, D], F32, tag="ao", name="ao")
                if bl:
                    for i, (g, ri) in enumerate(bl):
                        nc.tensor.matmul(
                            psa[:], st["eh"][i][:, t * 128 : (t + 1) * 128],
                            vo_sb[g][:], start=(i == 0), stop=False)
                nc.tensor.matmul(psa[:], identr_sb[:], hrow[t][:],
                                 start=(not bl), stop=True)
                nc.vector.bn_stats(out=st["stats"][:], in_=psa[:])
                nc.vector.bn_aggr(out=st["mvb"][:, t, :], in_=st["stats"][:])
                st["pg"][t] = psa

        def front_apply(b, st, c):
            """rsqrt + LN1 apply + transpose to xT for tiles 3c..3c+2."""
            t0 = c * 3
            mvb, sc1, ng1 = st["mvb"], st["sc1"], st["ng1"]
            rsqrt_group(mvb[:, t0 : t0 + 3, 1], mvb[:, t0 : t0 + 3, 0],
                        sc1[:, t0 : t0 + 3], ng1[:, t0 : t0 + 3],
                        st["sA"][:], st["sB"][:])
            for t in range(t0, t0 + 3):
                nc.scalar.activation(
                    st["x_row"][:, t, :], st["pg"].pop(t)[:], AF.Identity,
                    bias=ng1[:, t : t + 1], scale=sc1[:, t : t + 1])
                psx = ps_tp.tile([128, D], BF16, tag="tp", name="xp")
                for k in range(2):
                    nc.tensor.transpose(
                        psx[:, k * 128 : (k + 1) * 128],
                        st["x_row"][:, t, k * 128 : (k + 1) * 128],
                        identb_sb[:])
                nc.vector.tensor_copy(
                    out=st["xT"][:, :, t * 128 : (t + 1) * 128],
                    in_=psx[:].rearrange("p (k n) -> p k n", k=2))

        def back_ffn1(b, st, half):
            """FFN1 for 4 of the 8 m-chunks, both 384-chunks per stationary
            so each weight load is amortized over 768 moving columns."""
            if half == 0:
                st["h1g"] = work.tile([128, 8, NG], BF16, tag="h1g", name="h1g")
            xT = st["xT"]
            for m in range(half * 4, half * 4 + 4):
                pss = [ps_ff.tile([128, CH], F32, tag="ff", name="f1")
                       for _ in range(2)]
                for kk in range(2):
                    for c in range(NCH):
                        nc.tensor.matmul(
                            pss[c][:], w1_sb[kk][:, m * 128 : (m + 1) * 128],
                            xT[:, kk, c * CH : (c + 1) * CH],
                            start=(kk == 0), stop=(kk == 1))
                for c in range(NCH):
                    if use_b1:
                        nc.scalar.activation(
                            st["h1g"][:, m, c * CH : (c + 1) * CH],
                            pss[c][:], AF.Gelu, bias=b1_sb[m][:, 0:1])
                    else:
                        nc.scalar.activation(
                            st["h1g"][:, m, c * CH : (c + 1) * CH],
                            pss[c][:], AF.Gelu)

        def back_ffn2_ln2(b, st):
            # FFN2 (T layout) + x residual accumulated on the PE; ACT evac
            xres = st["xT"]
            if use_g1 or use_b1ln:
                xres = work.tile([128, 2, NG], BF16, tag="xres", name="xres")
                for k in range(2):
                    nc.vector.tensor_scalar(
                        out=xres[:, k, :], in0=st["xT"][:, k, :],
                        scalar1=ln1_sb[k][:, 0:1], scalar2=ln1_sb[k][:, 1:2],
                        op0=ALU.mult, op1=ALU.add)
            yT = work.tile([128, 2, NG], BF16, tag="yT", name="yT")
            for m in range(2):
                pss = [ps_ff.tile([128, CH], F32, tag="ff", name="y")
                       for _ in range(2)]
                for kk in range(8):
                    for c in range(NCH):
                        nc.tensor.matmul(
                            pss[c][:], w2_sb[kk][:, m * 128 : (m + 1) * 128],
                            st["h1g"][:, kk, c * CH : (c + 1) * CH],
                            start=(kk == 0), stop=False)
                for c in range(NCH):
                    nc.tensor.matmul(
                        pss[c][:], identb_sb[:],
                        xres[:, m, c * CH : (c + 1) * CH],
                        start=False, stop=True)
                    nc.scalar.activation(yT[:, m, c * CH : (c + 1) * CH],
                                         pss[c][:], AF.Copy)

            mv2 = work.tile([128, NT, 2], F32, tag="mv2", name="mv2")
            sc2 = work.tile([128, NT], F32, tag="sc2", name="sc2")
            ng2 = work.tile([128, NT], F32, tag="ng2", name="ng2")
            st2 = work.tile([128, 6], F32, tag="st2", name="st2")
            sA2 = work.tile([128, 3], F32, tag="sA2", name="sA2")
            sB2 = work.tile([128, 3], F32, tag="sB2", name="sB2")
            orow = work.tile([128, NT, D], F32, tag="orow", name="orow")
            for c in range(NCH):
                t0 = c * 3
                pg = []
                for t in range(t0, t0 + 3):
                    psy = ps_tp.tile([128, D], BF16, tag="tp", name="yt")
                    for k in range(2):
                        nc.tensor.transpose(
                            psy[:, k * 128 : (k + 1) * 128],
                            yT[:, k, t * 128 : (t + 1) * 128], identb_sb[:])
                    if use_b2:
                        nc.vector.tensor_add(psy[:], psy[:], gbr_sb[:, 0, :])
                    nc.vector.bn_stats(out=st2[:], in_=psy[:])
                    nc.vector.bn_aggr(out=mv2[:, t, :], in_=st2[:])
                    pg.append(psy)
                rsqrt_group(mv2[:, t0 : t0 + 3, 1], mv2[:, t0 : t0 + 3, 0],
                            sc2[:, t0 : t0 + 3], ng2[:, t0 : t0 + 3],
                            sA2[:], sB2[:])
                for t in range(t0, t0 + 3):
                    nc.scalar.activation(
                        orow[:, t, :], pg[t - t0][:], AF.Identity,
                        bias=ng2[:, t : t + 1], scale=sc2[:, t : t + 1])
                    if use_g2:
                        nc.vector.tensor_mul(orow[:, t, :], orow[:, t, :],
                                             gbr_sb[:, 2, :])
                    if use_b2ln:
                        nc.vector.tensor_add(orow[:, t, :], orow[:, t, :],
                                             gbr_sb[:, 3, :])
            outv = out[b if b is not None else 0].rearrange(
                "(t p) d -> p t d", p=128)
            if b is not None:
                dma(outv, orow[:])
            else:
                for be in range(B):
                    if int(counts[be]) == 0:
                        dma(out[be].rearrange("(t p) d -> p t d", p=128),
                            orow[:])

        alljobs = [(b, blocks[b]) for b in jobs]
        if any_empty:
            alljobs.append((None, []))
        prev = None
        for b, bl in alljobs:
            st = front_new(b)
            st["eh"] = front_attn(b, bl) if bl else []
            front_group(b, bl, st, 0)
            if prev is not None:
                back_ffn1(prev[0], prev[1], 0)
            front_apply(b, st, 0)
            front_group(b, bl, st, 1)
            if prev is not None:
                back_ffn1(prev[0], prev[1], 1)
            front_apply(b, st, 1)
            if prev is not None:
                back_ffn2_ln2(prev[0], prev[1])
            prev = (b, st)
        back_ffn1(prev[0], prev[1], 0)
        back_ffn1(prev[0], prev[1], 1)
        back_ffn2_ln2(prev[0], prev[1])

        cstack.close()

    return nc


def kernel(H_genes, perturbation_indices, batch_assignment, batch_size,
           in_proj_w, in_proj_b, out_proj_w, out_proj_b,
           ffn_w1, ffn_b1, ffn_w2, ffn_b2,
           ln1_g, ln1_b, ln2_g, ln2_b):
    import ml_dtypes
    bf16 = ml_dtypes.bfloat16

    Hg = np.ascontiguousarray(np.asarray(H_genes, dtype=np.float32))
    pidx = np.asarray(perturbation_indices).astype(np.int64)
    ba = np.asarray(batch_assignment).astype(np.int64)
    Bs = int(np.asarray(batch_size))
    assert Bs == B, f"kernel hardcodes B=16, got {Bs}"
    assert Hg.shape == (N, D)

    Wq, Wk, Wv = [np.asarray(w, np.float32) for w in np.split(np.asarray(in_proj_w), 3, axis=0)]
    bq, bk, bv = [np.asarray(x, np.float32) for x in np.split(np.asarray(in_proj_b), 3, axis=0)]
    Wo = np.asarray(out_proj_w, np.float32)
    bo = np.asarray(out_proj_b, np.float32)
    W1 = np.asarray(ffn_w1, np.float32)
    b1 = np.asarray(ffn_b1, np.float32)
    W2 = np.asarray(ffn_w2, np.float32)
    b2 = np.asarray(ffn_b2, np.float32)
    g1 = np.asarray(ln1_g, np.float32)
    be1 = np.asarray(ln1_b, np.float32)
    g2 = np.asarray(ln2_g, np.float32)
    be2 = np.asarray(ln2_b, np.float32)

    counts = np.bincount(ba, minlength=B).astype(np.int64)
    has_any = counts > 0

    # host-side small projections: k and Wo-folded values
    Hp = Hg[pidx]                                   # [P, D]
    k = Hp @ Wk.T + bk[None, :]                     # [P, D]
    v = Hp @ Wv.T + bv[None, :]                     # [P, D]
    # vo[(h,p),:] = v[p, h-slice] @ Wo[:, h-slice].T  (full attn_out proj)
    vo = np.zeros((NGRP, 128, D), np.float32)
    for g in range(NGRP):
        for h in range(H):
            vh = v[g * GW : (g + 1) * GW, h * DH : (h + 1) * DH]   # [16, 32]
            vo[g, h * GW : (h + 1) * GW, :] = vh @ Wo[:, h * DH : (h + 1) * DH].T

    # block-diagonal per-head mask matrices for denominators
    m01 = (ba[:, None] == np.arange(B)[None, :]).astype(np.float32)  # [P, B]
    m01bd = np.zeros((NGRP, 128, 128), np.float32)
    for g in range(NGRP):
        for h in range(H):
            m01bd[g, h * GW : (h + 1) * GW, h * GW : (h + 1) * GW] = \
                m01[g * GW : (g + 1) * GW, :]

    # replication matrices: rep[(h,b'),(h,p16)] = 1{b'==b and ba[p]==b}
    blocks = {b: [] for b in range(B)}
    rep_mats = []
    for b in range(B):
        if counts[b] == 0:
            continue
        for g in range(NGRP):
            psel = np.where(ba[g * GW : (g + 1) * GW] == b)[0]
            if len(psel) == 0:
                continue
            R = np.zeros((128, 128), np.float32)
            for h in range(H):
                for p16 in psel:
                    R[h * GW + b, h * GW + p16] = 1.0
            blocks[b].append((g, len(rep_mats)))
            rep_mats.append(R)
    nrep = len(rep_mats)

    # fold ln1 gain into FFN1 (exact): W1' = W1*g1, b1' = W1@b1_ln + b1
    W1f = W1 * g1[None, :]
    b1f = b1 + W1 @ be1

    Hg_pad = np.zeros((NPAD, D), np.float32)
    Hg_pad[:N] = Hg
    emptyp = np.tile((~has_any).astype(np.float32), H)[:, None]  # [(h,b),1]

    flags = (
        bool(np.any(bq != 0)), bool(np.any(b1f != 0)), bool(np.any(b2 != 0)),
        bool(np.any(g1 != 1)), bool(np.any(be1 != 0)),
        bool(np.any(g2 != 1)), bool(np.any(be2 != 0)),
    )
    use_bo = bool(np.any(bo != 0))

    nc = _build_program(counts, blocks, nrep, flags)

    common = {
        "kt": np.ascontiguousarray(k.T),
        "wq_t": np.ascontiguousarray(Wq.T),
        "bq_col": bq[:, None].copy(),
        "vo": vo.astype(bf16),
        "m01bd": m01bd.astype(bf16),
        "repm": (np.stack(rep_mats) if nrep else np.zeros((1, 128, 128), np.float32)).astype(bf16),
        "identb": np.eye(128, dtype=np.float32).astype(bf16),
        "identf": np.eye(128, dtype=np.float32),
        "identr": np.eye(128, dtype=np.float32),
        "emptyp": np.ascontiguousarray(emptyp),
        "w1_t": np.ascontiguousarray(W1f.T).astype(bf16),
        "w2_t": np.ascontiguousarray(W2.T).astype(bf16),
        "b1_col": b1f[:, None].copy(),
        "ln1_col": np.ascontiguousarray(np.stack([g1, be1], axis=1)),
        "gb_row": np.stack([b2, be1, g2, be2], axis=0),
    }
    in_maps = []
    for c in range(NCORES):
        sl = Hg_pad[c * NG : (c + 1) * NG]
        m = dict(common)
        m["hg_row"] = np.ascontiguousarray(sl)
        m["hgb_row"] = np.ascontiguousarray(sl + bo[None, :]) if use_bo else m["hg_row"]
        m["hg_t"] = np.ascontiguousarray(sl.T)
        in_maps.append(m)

    if os.environ.get("BASS_KERNEL_SIM"):
        from concourse import bass_interp
        # CoreSim lacks a Gelu LUT; shim exact (erf) gelu for local debugging.
        if not getattr(bass_interp.InstructionExecutor, "_gelu_patched", False):
            from scipy.special import erf
            _orig_act = bass_interp.InstructionExecutor.visit_InstActivation

            def _act(self, instruction, *, reg_snapshot=None):
                if instruction.func == mybir.ActivationFunctionType.Gelu:
                    instruction.func = mybir.ActivationFunctionType.Identity
                    try:
                        import concourse.bass_interp as bi
                        out_ap = instruction.outs[0]
                        r = _orig_act(self, instruction, reg_snapshot=reg_snapshot)
                        view = self.view_ap(out_ap, bi.Direction.READ, instruction,
                                            reg_snapshot=reg_snapshot)
                        x = view.astype(np.float64)
                        view[:] = (0.5 * x * (1.0 + erf(x / np.sqrt(2.0)))).astype(view.dtype)
                        return r
                    finally:
                        instruction.func = mybir.ActivationFunctionType.Gelu
                return _orig_act(self, instruction, reg_snapshot=reg_snapshot)

            bass_interp.InstructionExecutor.visit_InstActivation = _act
            bass_interp.InstructionExecutor._gelu_patched = True
        nsim = int(os.environ.get("BASS_KERNEL_SIM_CORES", "1"))
        simtrace = bool(os.environ.get("BASS_KERNEL_SIMTRACE"))
        sim = bass_interp.MultiCoreSim(nc, nsim, trace=simtrace)
        for c in range(nsim):
            for kk, vv in in_maps[c].items():
                sim.cores[c].tensor(kk)[:] = vv
        sim.simulate()
        print(f"SIM predicted time: {sim.cores[0].time} ns")
        full = np.zeros((B, NPAD, D), np.float32)
        for c in range(nsim):
            full[:, c * NG : (c + 1) * NG, :] = (
                np.array(sim.cores[c].mem_tensor("out")).reshape(B, NG, D))
        return full[:, :N, :]

    from concourse.bass_utils import run_bass_kernel_spmd
    _split_waits(nc)
    trace = bool(os.environ.get("BASS_KERNEL_TRACE"))
    res = run_bass_kernel_spmd(nc, in_maps, core_ids=list(range(NCORES)),
                               trace=trace)
    if trace and res.exec_time_ns is not None:
        print(f"HW exec time: {res.exec_time_ns} ns")
        if res.instructions_and_trace:
            print("trace:", res.instructions_and_trace[1])

    full = np.zeros((B, NPAD, D), np.float32)
    for c in range(NCORES):
        full[:, c * NG : (c + 1) * NG, :] = res.results[c]["out"]
    return full[:, :N, :]
